# revision 1
# baseline (speedup 1.0000x reference)
"""HMM forward-backward (batch=256, seq=512, Z=64) on 8 Trainium2 NeuronCores.

Strategy (data parallel over batch, 32 batch elements per core):
  - Emission rows e[t,b,:] = emit[input[t,b]] are gathered on-device with
    dma_gather (int16 indices), landing in natural [row=(t,b), Z] layout,
    then PE-transposed to a [Z, (t,b)] layout ("E2").
  - Forward and backward recursions are merged into ONE 128-contraction
    matmul per step with a block-diagonal stationary matrix
    W = diag(T, T^T):  state = [v_{S-1-q} (rows 0:64) ; alpha_q (rows 64:128)]
    per column group q.  One DVE multiply with the E2 column produces the
    next state column.  beta_{S-2-j} is the top PSUM half before the
    multiply and is copied off by the Scalar engine.
  - posterior = (alpha*beta) normalized per (t,b) over Z: Z lives on the
    partition dim, so the column sums use a ones-vector matmul, then
    reciprocal + gpsimd partition_broadcast + DVE multiplies.
  - Outputs are produced in [Z, t*32+b] layout per core; the host
    reassembles/transposes to [S, B, Z] (pure numpy layout work).
"""

import sys

for _p in ("/opt/trn_rl_repo", "/root/.axon_site/_ro/trn_rl_repo"):
    if _p not in sys.path:
        sys.path.append(_p)

import numpy as np

import concourse.bacc as bacc
import concourse.mybir as mybir
from concourse.bass_utils import run_bass_kernel_spmd
from concourse.tile import TileContext

S = 512          # sequence length
B = 256          # total batch
Z = 64           # hidden states
X = 10000        # emission vocab
NCORES = 8
Bc = B // NCORES           # batch per core = 32
COLS = S * Bc              # 16384 state columns per core
CH = 64                    # timesteps per gather/E2 chunk
CCOLS = CH * Bc            # 2048 columns per chunk
NCH = S // CH              # 8 chunks
NBLK = CCOLS // 128        # 16 transpose blocks (of 4 timesteps) per chunk
PCOLS = 1024               # posterior chunk columns (32 timesteps)
NPCH = COLS // PCOLS       # 16 posterior chunks

F32 = mybir.dt.float32
MUL = mybir.AluOpType.mult

_CACHE = {}
LAST_RESULTS = None


def _build_nc():
    nc = bacc.Bacc("TRN2", target_bir_lowering=False, debug=False,
                   num_devices=NCORES)

    emit_d = nc.dram_tensor("emit", [X, Z], F32, kind="ExternalInput")
    idxc_d = nc.dram_tensor("idxc", [128, 2 * COLS // 16], mybir.dt.int16,
                            kind="ExternalInput")
    w_d = nc.dram_tensor("w", [128, 128], F32, kind="ExternalInput")
    id_d = nc.dram_tensor("ident", [128, 128], F32, kind="ExternalInput")
    pi_d = nc.dram_tensor("piext", [128, 1], F32, kind="ExternalInput")

    alpha_d = nc.dram_tensor("alpha", [64, COLS], F32, kind="ExternalOutput")
    beta_d = nc.dram_tensor("beta", [64, COLS], F32, kind="ExternalOutput")
    post_d = nc.dram_tensor("post", [64, COLS], F32, kind="ExternalOutput")

    with TileContext(nc) as tc:
        with (
            tc.tile_pool(name="const", bufs=1) as constp,
            tc.tile_pool(name="state", bufs=1) as statep,
            tc.tile_pool(name="betap", bufs=1) as betapp,
            tc.tile_pool(name="e2", bufs=2) as e2p,
            tc.tile_pool(name="gst", bufs=3) as gstp,
            tc.tile_pool(name="ab", bufs=2) as abp,
            tc.tile_pool(name="bc", bufs=2) as bcp,
            tc.tile_pool(name="po", bufs=2) as pop,
            tc.tile_pool(name="rec", bufs=2) as recp,
            tc.tile_pool(name="mm", bufs=4, space="PSUM") as mmp,
            tc.tile_pool(name="tr", bufs=4, space="PSUM") as trp,
        ):
            # ---- constants ----
            idxc_t = constp.tile([128, 2 * COLS // 16], mybir.dt.int16,
                                 tag="idxc")
            w_t = constp.tile([128, 128], F32, tag="w")
            id_t = constp.tile([128, 128], F32, tag="id")
            pi_t = constp.tile([128, 1], F32, tag="pi")
            nc.sync.dma_start(idxc_t[:], idxc_d[:])
            nc.sync.dma_start(w_t[:], w_d[:])
            nc.sync.dma_start(id_t[:], id_d[:])
            nc.sync.dma_start(pi_t[:], pi_d[:])

            state = statep.tile([128, COLS], F32, tag="state")
            betap = betapp.tile([128, COLS], F32, tag="beta")  # rows 64:128

            gtiles = {}   # (chunk, which 0=bwd/top 1=fwd/bottom) -> tile
            e2tiles = {}  # chunk -> tile

            # dma_gather is limited to ~1024 indices per instruction.
            # One interleaved gather per chunk: even blocks = bwd rows,
            # odd blocks = fwd rows -> [128, 2*NBLK, 64] staging.
            GI = 1024
            GSPLIT = 2 * CCOLS // GI  # 4 gathers per chunk

            def issue_gather(c):
                g = gstp.tile([128, 2 * NBLK, Z], F32, tag="g", name=f"g_{c}")
                for h in range(GSPLIT):
                    nb = GI // 128
                    nc.gpsimd.dma_gather(
                        g[:, h * nb:(h + 1) * nb, :], emit_d[:],
                        idxc_t[:, c * (2 * CCOLS // 16) + h * (GI // 16):
                               c * (2 * CCOLS // 16) + (h + 1) * (GI // 16)],
                        GI, GI, Z)
                gtiles[c] = g

            def transpose_pair(c, m):
                if m == 0:
                    e2tiles[c] = e2p.tile([128, CCOLS], F32, tag="e2", name=f"e2_{c}")
                e2 = e2tiles[c]
                pA = trp.tile([128, 128], F32, tag="tr")
                gv = gtiles[c][:, 2 * m:2 * m + 2, :]
                nc.tensor.transpose(pA[:], gv.rearrange("p a b -> p (a b)"),
                                    id_t[:])
                cs = slice(m * 128, (m + 1) * 128)
                nc.scalar.copy(e2[:, cs], pA[:])

            # ---- prologue ----
            issue_gather(0)
            issue_gather(1)
            for m in range(NBLK):
                transpose_pair(0, m)
            # state col 0 = E2 col 0 * [ones; pi]
            nc.vector.tensor_scalar(state[:, 0:Bc], e2tiles[0][:, 0:Bc],
                                    pi_t[:, 0:1], None, MUL)
            # beta[S-1] = 1
            nc.vector.memset(betap[64:128, (S - 1) * Bc:S * Bc], 1.0)

            # ---- posterior machinery: sliced ops so they fit engine idle
            # windows inside the scan without stretching the chain ----
            PSL = 256                       # posterior slice columns
            NSL = PCOLS // PSL              # 4 slices per chunk

            def post_ops(p):
                """Closures computing posterior chunk p, in dependency order,
                each small enough to hide in per-step engine idle time."""
                cs = slice(p * PCOLS, (p + 1) * PCOLS)
                ctx = {}
                ops = []

                def mk_ab(k):
                    def fn():
                        if k == 0:
                            ctx["ab"] = abp.tile([64, PCOLS], F32, tag="ab",
                                                 name=f"ab_{p}")
                        s = slice(p * PCOLS + k * PSL, p * PCOLS + (k + 1) * PSL)
                        nc.vector.tensor_tensor(
                            ctx["ab"][:, k * PSL:(k + 1) * PSL],
                            state[64:128, s], betap[64:128, s], MUL)
                    return fn

                def mk_sum(k):
                    def fn():
                        if k == 0:
                            ctx["rec"] = recp.tile([1, PCOLS], F32, tag="rec",
                                                   name=f"rec_{p}")
                        pssum = trp.tile([1, PSL], F32, tag="tr",
                                         name=f"pss_{p}_{k}")
                        nc.tensor.matmul(pssum[:], pi_t[0:64, 0:1],
                                         ctx["ab"][:, k * PSL:(k + 1) * PSL])
                        nc.vector.reciprocal(
                            ctx["rec"][:, k * PSL:(k + 1) * PSL], pssum[:])
                    return fn

                def mk_bcast():
                    def fn():
                        ctx["bc"] = bcp.tile([64, PCOLS], F32, tag="bc",
                                             name=f"bc_{p}")
                        nc.gpsimd.partition_broadcast(ctx["bc"][:],
                                                      ctx["rec"][:, :])
                    return fn

                def mk_pm(k):
                    def fn():
                        if k == 0:
                            ctx["po"] = pop.tile([64, PCOLS], F32, tag="po",
                                                 name=f"po_{p}")
                        nc.vector.tensor_tensor(
                            ctx["po"][:, k * PSL:(k + 1) * PSL],
                            ctx["ab"][:, k * PSL:(k + 1) * PSL],
                            ctx["bc"][:, k * PSL:(k + 1) * PSL], MUL)
                        if k == NSL - 1:
                            nc.sync.dma_start(post_d[:, cs], ctx["po"][:])
                    return fn

                for k in range(NSL):
                    ops.append(mk_ab(k))
                for k in range(NSL):
                    ops.append(mk_sum(k))
                ops.append(mk_bcast())
                for k in range(NSL):
                    ops.append(mk_pm(k))
                return ops

            # chunks whose alpha+beta are ready mid-scan: p=8..14 inline,
            # two ops every step starting 2 steps after alpha lands
            TAIL_PCH = [15, 0]
            POST_SCHED = {}
            for p in range(8, 15):          # alpha-bound: ready after 32p+31
                for i, fn in enumerate(post_ops(p)):
                    POST_SCHED.setdefault(32 * p + 33 + 2 * i, []).append(fn)
            for p in range(1, 8):           # beta-bound: ready after 510-32p
                for i, fn in enumerate(post_ops(p)):
                    POST_SCHED.setdefault(512 - 32 * p + 2 * i, []).append(fn)

            # transpose emission schedule: pair m of chunk c+1 is emitted
            # inside chunk c at step offset TR_SLOTS[m]
            TR_SLOTS = {4 + 3 * m: m for m in range(NBLK)}

            # ---- merged forward/backward scan ----
            for j in range(S - 1):
                c, off = j // CH, j % CH
                if off == 0 and c + 2 < NCH:
                    issue_gather(c + 2)
                if off in TR_SLOTS and c + 1 < NCH:
                    transpose_pair(c + 1, TR_SLOTS[off])

                ps = mmp.tile([128, Bc], F32, tag="mm")
                nc.tensor.matmul(ps[:], w_t[:], state[:, j * Bc:(j + 1) * Bc])
                q = j + 1
                nc.vector.tensor_tensor(
                    state[:, q * Bc:(q + 1) * Bc], ps[:],
                    e2tiles[q // CH][:, (q % CH) * Bc:((q % CH) + 1) * Bc],
                    MUL)
                tb = S - 2 - j
                nc.scalar.copy(betap[64:128, tb * Bc:(tb + 1) * Bc],
                               ps[0:64, :])
                # stream completed output chunks out under the scan
                if off == CH - 1 and c < NCH - 1:       # alpha chunk c done
                    acs = slice(c * CCOLS, (c + 1) * CCOLS)
                    nc.sync.dma_start(alpha_d[:, acs], state[64:128, acs])
                bc_ = (S - 1 - j) // CH                 # beta chunk bc_ done when j == 511-64*bc_
                if bc_ >= 1 and j == (S - 1) - CH * bc_ and bc_ <= NCH - 1:
                    bcs = slice(bc_ * CCOLS, (bc_ + 1) * CCOLS)
                    nc.sync.dma_start(beta_d[:, bcs], betap[64:128, bcs])
                for fn in POST_SCHED.get(j, []):
                    fn()

            # ---- outputs: alpha/beta straight out, posterior normalized ----
            cs = slice((NCH - 1) * CCOLS, NCH * CCOLS)
            nc.sync.dma_start(alpha_d[:, cs], state[64:128, cs])
            cs = slice(0, CCOLS)
            nc.sync.dma_start(beta_d[:, cs], betap[64:128, cs])

            for p in TAIL_PCH:
                for fn in post_ops(p):
                    fn()

    nc.finalize()
    return nc


def _wrap_idx(lin):
    """Linear index list -> [128, N//16] int16 gather-index layout
    (position i at [i % 16, i // 16], replicated over partition groups)."""
    n = lin.shape[0]
    w = lin.reshape(n // 16, 16).T.astype(np.int16)   # [16, n//16]
    return np.tile(w, (8, 1))


def kernel(input, T, pi, emit):
    global LAST_RESULTS
    input = np.asarray(input)
    T = np.asarray(T, dtype=np.float32)
    pi = np.asarray(pi, dtype=np.float32)
    emit = np.asarray(emit, dtype=np.float32)

    if "nc" not in _CACHE:
        _CACHE["nc"] = _build_nc()
    nc = _CACHE["nc"]

    W = np.zeros((128, 128), np.float32)
    W[:64, :64] = T          # backward block: out_top = T^T @ v
    W[64:, 64:] = T.T        # forward block:  out_bot = T @ alpha
    pi_ext = np.ones((128, 1), np.float32)
    pi_ext[64:, 0] = pi
    ident = np.eye(128, dtype=np.float32)

    in_maps = []
    for c in range(NCORES):
        sl = input[:, c * Bc:(c + 1) * Bc].astype(np.int64)   # [S, Bc]
        lin_f = sl.reshape(-1)                                # i = t*Bc+b
        lin_b = sl[::-1, :].reshape(-1)                       # i = k*Bc+b, t=S-1-k
        # interleave 128-row blocks: [bwd m, fwd m] per block pair
        fb = lin_b.reshape(-1, 128)                           # [128 blocks, 128]
        ff = lin_f.reshape(-1, 128)
        lin_c = np.stack([fb, ff], axis=1).reshape(-1)        # [2*COLS]
        in_maps.append({
            "emit": emit,
            "idxc": _wrap_idx(lin_c),
            "w": W,
            "ident": ident,
            "piext": pi_ext,
        })

    res = run_bass_kernel_spmd(nc, in_maps, core_ids=list(range(NCORES)))
    LAST_RESULTS = res

    alpha = np.empty((S, B, Z), np.float32)
    beta = np.empty((S, B, Z), np.float32)
    post = np.empty((S, B, Z), np.float32)
    for c in range(NCORES):
        r = res.results[c]
        bs = slice(c * Bc, (c + 1) * Bc)
        alpha[:, bs, :] = r["alpha"].reshape(Z, S, Bc).transpose(1, 2, 0)
        beta[:, bs, :] = r["beta"].reshape(Z, S, Bc).transpose(1, 2, 0)
        post[:, bs, :] = r["post"].reshape(Z, S, Bc).transpose(1, 2, 0)
    return alpha, beta, post



# revision 6
# speedup vs baseline: 1.1021x; 1.1021x over previous
"""HMM forward-backward (batch=256, seq=512, Z=64) on 8 Trainium2 NeuronCores.

Strategy (data parallel over batch, 32 batch elements per core):
  - Emission rows are pre-gathered ON HOST into the merged layout
    e2[128, S*Bc]: rows 0:64 = emit[input[S-1-k,b]] (backward, time-reversed),
    rows 64:128 = emit[input[k,b]] (forward); column index = k*Bc+b.
    The device streams e2 per 64-step chunk via plain HWDGE DMA (no
    gathers, no PE transposes) in 256-column slices for fine-grained deps.
  - Forward and backward recursions are merged into ONE 128-contraction
    matmul per step with a block-diagonal stationary matrix
    W = diag(T, T^T):  state = [v_{S-1-q} (rows 0:64) ; alpha_q (rows 64:128)]
    per column group q.  One DVE multiply with the e2 column produces the
    next state column.  beta_{S-2-j} is the top PSUM half before the
    multiply and is copied off by the Scalar engine.
    Critical path per step: PE matmul -> DVE tensor_tensor -> PE.
  - posterior = alpha*beta / L where L = sum_z alpha_t*beta_t is CONSTANT
    over t (HMM likelihood identity).  L is computed once mid-scan
    (one ones-vector matmul + reciprocal), broadcast/tiled once, and the
    per-chunk posterior multiplies run on the otherwise-idle GPSIMD (Pool)
    engine so they never touch the PE/DVE critical path.  The two chunks
    whose alpha/beta only complete at scan end are processed in small
    slivers during the last steps to minimize the tail.
  - Outputs are produced in [Z, t*32+b] layout per core; the host
    reassembles/transposes to [S, B, Z] (pure numpy layout work).
"""

import sys

for _p in ("/opt/trn_rl_repo", "/root/.axon_site/_ro/trn_rl_repo"):
    if _p not in sys.path:
        sys.path.append(_p)

import numpy as np

import concourse.bacc as bacc
import concourse.mybir as mybir
from concourse.bass_utils import run_bass_kernel_spmd
from concourse.tile import TileContext

S = 512          # sequence length
B = 256          # total batch
Z = 64           # hidden states
NCORES = 8
Bc = B // NCORES           # batch per core = 32
COLS = S * Bc              # 16384 state columns per core
CH = 64                    # timesteps per e2 chunk
CCOLS = CH * Bc            # 2048 columns per chunk
NCH = S // CH              # 8 chunks
DSL = 256                  # e2 DMA slice columns (8 per chunk)
PCH = 32                   # timesteps per posterior chunk
PCOLS = PCH * Bc           # 1024 posterior chunk columns
NPCH = COLS // PCOLS       # 16 posterior chunks

F32 = mybir.dt.float32
MUL = mybir.AluOpType.mult

_CACHE = {}
LAST_RESULTS = None


def _build_nc():
    nc = bacc.Bacc("TRN2", target_bir_lowering=False, debug=False,
                   num_devices=NCORES)

    e2_d = nc.dram_tensor("e2", [128, COLS], F32, kind="ExternalInput")
    w_d = nc.dram_tensor("w", [128, 128], F32, kind="ExternalInput")
    pi_d = nc.dram_tensor("piext", [128, 1], F32, kind="ExternalInput")
    ones_d = nc.dram_tensor("ones64", [64, 64], F32, kind="ExternalInput")

    alpha_d = nc.dram_tensor("alpha", [64, COLS], F32, kind="ExternalOutput")
    beta_d = nc.dram_tensor("beta", [64, COLS], F32, kind="ExternalOutput")
    post_d = nc.dram_tensor("post", [64, COLS], F32, kind="ExternalOutput")

    with TileContext(nc) as tc:
        with (
            tc.tile_pool(name="const", bufs=1) as constp,
            tc.tile_pool(name="state", bufs=1) as statep,
            tc.tile_pool(name="betap", bufs=1) as betapp,
            tc.tile_pool(name="linv", bufs=1) as linvp,
            tc.tile_pool(name="e2", bufs=2) as e2p,
            tc.tile_pool(name="po", bufs=2) as pop,
            tc.tile_pool(name="pot", bufs=2) as potp,
            tc.tile_pool(name="mm", bufs=4, space="PSUM") as mmp,
            tc.tile_pool(name="aux", bufs=2, space="PSUM") as auxp,
        ):
            # ---- constants ----
            w_t = constp.tile([128, 128], F32, tag="w")
            pi_t = constp.tile([128, 1], F32, tag="pi")
            ones_t = constp.tile([64, 64], F32, tag="ones")
            nc.sync.dma_start(w_t[:], w_d[:])
            nc.sync.dma_start(pi_t[:], pi_d[:])
            nc.sync.dma_start(ones_t[:], ones_d[:])

            state = statep.tile([128, COLS], F32, tag="state")
            betap = betapp.tile([128, COLS], F32, tag="beta")  # rows 64:128 used
            linvt = linvp.tile([64, CCOLS], F32, tag="linv")

            e2tiles = {}

            def issue_e2(c):
                """Stream e2 chunk c from DRAM in DSL-column slices."""
                t = e2p.tile([128, CCOLS], F32, tag="e2", name=f"e2_{c}")
                e2tiles[c] = t
                base = c * CCOLS
                for h in range(CCOLS // DSL):
                    nc.sync.dma_start(
                        t[:, h * DSL:(h + 1) * DSL],
                        e2_d[:, base + h * DSL:base + (h + 1) * DSL])

            # ---- prologue ----
            issue_e2(0)
            issue_e2(1)
            # state col 0 = e2 col 0 * [ones; pi]
            nc.vector.tensor_scalar(state[:, 0:Bc], e2tiles[0][:, 0:Bc],
                                    pi_t[:, 0:1], None, MUL)
            # beta[S-1] = 1
            nc.gpsimd.memset(betap[64:128, (S - 1) * Bc:S * Bc], 1.0)

            # ---- posterior machinery ----
            # L_b = sum_z alpha_t[z,b]*beta_t[z,b] is t-independent; computed
            # at the mid-scan step from column group MIDQ.  Pool computes
            # ab = alpha.*beta for that column group, PE reduces over z with
            # a ones-stationary matmul into 64 identical rows, DVE
            # reciprocates -> linv tile rows, Pool tiles it across CCOLS.
            MIDQ = S // 2 - 1          # both alpha_q and beta_q exist then

            def post_chunk_ops(p):
                """Closures for posterior chunk p (Pool TTs + output DMA),
                sliced in SLC-column pieces."""
                SLC = 512
                ops = []
                po = {}

                def mk(i):
                    def fn():
                        if i == 0:
                            po["t"] = pop.tile([64, PCOLS], F32, tag="po",
                                               name=f"po_{p}")
                        s = slice(p * PCOLS + i * SLC,
                                  p * PCOLS + (i + 1) * SLC)
                        d = slice(i * SLC, (i + 1) * SLC)
                        nc.gpsimd.tensor_tensor(po["t"][:, d], state[64:128, s],
                                                betap[64:128, s], MUL)
                        nc.gpsimd.tensor_tensor(
                            po["t"][:, d], po["t"][:, d],
                            linvt[:, i * SLC % CCOLS:
                                  (i * SLC % CCOLS) + SLC], MUL)
                        if i == PCOLS // SLC - 1:
                            nc.sync.dma_start(
                                post_d[:, p * PCOLS:(p + 1) * PCOLS],
                                po["t"][:])
                    return fn

                for i in range(PCOLS // SLC):
                    ops.append(mk(i))
                return ops

            # Posterior schedule: chunk p needs alpha through t=32p+31
            # (ready after step j=32p+31) and beta through t=32p
            # (beta_tb written at step j=510-tb -> ready after j=510-32p).
            # Both ready at j >= max(32p+31, 510-32p); also needs linv
            # (ready ~ MIDQ+6).  Chunks 0 and 15 are handled by slivers.
            POST_SCHED = {}
            for p in range(1, NPCH - 1):
                j0 = max(32 * p + 33, 512 - 32 * p, MIDQ + 10)
                for i, fn in enumerate(post_chunk_ops(p)):
                    POST_SCHED.setdefault(j0 + 3 * i, []).append(fn)

            # Sliver schedule for chunks 0 (beta arrives last) and 15
            # (alpha arrives last): process 8 timesteps (256 cols) per
            # sliver as soon as their data lands.
            SLV = 8 * Bc
            po_tail = {}

            def sliver(p, k):
                """Posterior for cols [p*PCOLS + k*SLV : +SLV]."""
                def fn():
                    if p not in po_tail:
                        po_tail[p] = potp.tile([64, PCOLS], F32, tag="pot",
                                               name=f"pot_{p}")
                    s = slice(p * PCOLS + k * SLV, p * PCOLS + (k + 1) * SLV)
                    d = slice(k * SLV, (k + 1) * SLV)
                    nc.gpsimd.tensor_tensor(po_tail[p][:, d],
                                            state[64:128, s],
                                            betap[64:128, s], MUL)
                    nc.gpsimd.tensor_tensor(
                        po_tail[p][:, d], po_tail[p][:, d],
                        linvt[:, (p * PCOLS + k * SLV) % CCOLS:
                              (p * PCOLS + k * SLV) % CCOLS + SLV], MUL)
                return fn

            # chunk 15 sliver k: alpha cols up to t=480+8k+7 -> after step
            # j=487+8k.  beta chunk 15 (t=480..511) ready by j=30 (early).
            for k in range(4):
                POST_SCHED.setdefault(488 + 8 * k, []).append(sliver(15, k))
            # chunk 0 sliver k: beta cols t=8k..8k+7 -> beta_tb at step
            # 510-tb -> ready after j=510-8k; alpha chunk 0 early (j=31).
            for k in range(3, -1, -1):
                POST_SCHED.setdefault(504 - 8 * k, []).append(sliver(0, k))

            # ---- merged forward/backward scan ----
            for j in range(S - 1):
                c, off = j // CH, j % CH
                if off == 0 and c + 2 < NCH:
                    issue_e2(c + 2)

                ps = mmp.tile([128, Bc], F32, tag="mm")
                nc.tensor.matmul(ps[:], w_t[:], state[:, j * Bc:(j + 1) * Bc])
                q = j + 1
                nc.vector.tensor_tensor(
                    state[:, q * Bc:(q + 1) * Bc], ps[:],
                    e2tiles[q // CH][:, (q % CH) * Bc:((q % CH) + 1) * Bc],
                    MUL)
                tb = S - 2 - j
                nc.scalar.copy(betap[64:128, tb * Bc:(tb + 1) * Bc], ps[0:64, :])

                # mid-scan normalizer: L columns from group MIDQ
                if j == MIDQ + 2:
                    abm = pop.tile([64, Bc], F32, tag="po", name="abmid")
                    nc.gpsimd.tensor_tensor(
                        abm[:], state[64:128, MIDQ * Bc:(MIDQ + 1) * Bc],
                        betap[64:128, MIDQ * Bc:(MIDQ + 1) * Bc], MUL)
                elif j == MIDQ + 4:
                    lsum = auxp.tile([64, Bc], F32, tag="aux", name="lsum")
                    nc.tensor.matmul(lsum[:], ones_t[:], abm[:])
                elif j == MIDQ + 6:
                    nc.vector.reciprocal(linvt[:, 0:Bc], lsum[:])
                elif j == MIDQ + 8:
                    # tile linv [64, Bc] -> [64, CCOLS] by doubling
                    w_ = Bc
                    while w_ < CCOLS:
                        nc.gpsimd.tensor_copy(linvt[:, w_:min(2 * w_, CCOLS)],
                                              linvt[:, 0:min(w_, CCOLS - w_)])
                        w_ *= 2

                # stream completed alpha/beta chunks out under the scan
                if off == CH - 1 and c < NCH - 1:       # alpha chunk c done
                    acs = slice(c * CCOLS, (c + 1) * CCOLS)
                    nc.sync.dma_start(alpha_d[:, acs], state[64:128, acs])
                bc_ = (S - 1 - j) // CH
                if bc_ >= 1 and j == (S - 1) - CH * bc_ and bc_ <= NCH - 1:
                    bcs = slice(bc_ * CCOLS, (bc_ + 1) * CCOLS)
                    nc.sync.dma_start(beta_d[:, bcs], betap[64:128, bcs])
                for fn in POST_SCHED.get(j, []):
                    fn()

            # ---- epilogue: final output streaming ----
            cs = slice((NCH - 1) * CCOLS, NCH * CCOLS)
            nc.sync.dma_start(alpha_d[:, cs], state[64:128, cs])
            cs = slice(0, CCOLS)
            nc.sync.dma_start(beta_d[:, cs], betap[64:128, cs])
            nc.sync.dma_start(post_d[:, 15 * PCOLS:16 * PCOLS], po_tail[15][:])
            nc.sync.dma_start(post_d[:, 0:PCOLS], po_tail[0][:])

    nc.finalize()
    return nc


def kernel(input, T, pi, emit):
    global LAST_RESULTS
    input = np.asarray(input)
    T = np.asarray(T, dtype=np.float32)
    pi = np.asarray(pi, dtype=np.float32)
    emit = np.asarray(emit, dtype=np.float32)

    if "nc" not in _CACHE:
        _CACHE["nc"] = _build_nc()
    nc = _CACHE["nc"]

    W = np.zeros((128, 128), np.float32)
    W[:64, :64] = T          # backward block: out_top = T^T @ v
    W[64:, 64:] = T.T        # forward block:  out_bot = T @ alpha
    pi_ext = np.ones((128, 1), np.float32)
    pi_ext[64:, 0] = pi
    ones64 = np.ones((64, 64), np.float32)

    in_maps = []
    for c in range(NCORES):
        sl = np.asarray(input[:, c * Bc:(c + 1) * Bc], dtype=np.int64)
        ef = emit[sl]                              # [S, Bc, Z]
        e2 = np.empty((128, COLS), np.float32)
        e2[64:128] = ef.transpose(2, 0, 1).reshape(Z, COLS)
        e2[0:64] = ef[::-1].transpose(2, 0, 1).reshape(Z, COLS)
        in_maps.append({
            "e2": e2,
            "w": W,
            "piext": pi_ext,
            "ones64": ones64,
        })

    res = run_bass_kernel_spmd(nc, in_maps, core_ids=list(range(NCORES)))
    LAST_RESULTS = res

    alpha = np.empty((S, B, Z), np.float32)
    beta = np.empty((S, B, Z), np.float32)
    post = np.empty((S, B, Z), np.float32)
    for c in range(NCORES):
        r = res.results[c]
        bs = slice(c * Bc, (c + 1) * Bc)
        alpha[:, bs, :] = r["alpha"].reshape(Z, S, Bc).transpose(1, 2, 0)
        beta[:, bs, :] = r["beta"].reshape(Z, S, Bc).transpose(1, 2, 0)
        post[:, bs, :] = r["post"].reshape(Z, S, Bc).transpose(1, 2, 0)
    return alpha, beta, post


# revision 7
# speedup vs baseline: 1.1072x; 1.0047x over previous
"""HMM forward-backward (batch=256, seq=512, Z=64) on 8 Trainium2 NeuronCores.

Strategy (data parallel over batch, 32 batch elements per core):
  - Emission rows are pre-gathered ON HOST into the merged layout
    e2[128, S*Bc]: rows 0:64 = emit[input[S-1-k,b]] (backward, time-reversed),
    rows 64:128 = emit[input[k,b]] (forward); column index = k*Bc+b.
    The device streams e2 per 64-step chunk via plain HWDGE DMA (no
    gathers, no PE transposes) in 256-column slices for fine-grained deps.
  - Forward and backward recursions are merged into ONE 128-contraction
    matmul per step with a block-diagonal stationary matrix
    W = diag(T, T^T):  state = [v_{S-1-q} (rows 0:64) ; alpha_q (rows 64:128)]
    per column group q.  One DVE multiply with the e2 column produces the
    next state column.  beta_{S-2-j} is the top PSUM half before the
    multiply and is copied off by the Scalar engine.
    Critical path per step: PE matmul -> DVE tensor_tensor -> PE.
  - posterior = alpha*beta / L where L = sum_z alpha_t*beta_t is CONSTANT
    over t (HMM likelihood identity).  L is computed once mid-scan
    (one ones-vector matmul + reciprocal), broadcast/tiled once, and the
    per-chunk posterior multiplies run on the otherwise-idle GPSIMD (Pool)
    engine so they never touch the PE/DVE critical path.  The two chunks
    whose alpha/beta only complete at scan end are processed in small
    slivers during the last steps to minimize the tail.
  - Outputs are produced in [Z, t*32+b] layout per core; the host
    reassembles/transposes to [S, B, Z] (pure numpy layout work).
"""

import sys

for _p in ("/opt/trn_rl_repo", "/root/.axon_site/_ro/trn_rl_repo"):
    if _p not in sys.path:
        sys.path.append(_p)

import numpy as np

import concourse.bacc as bacc
import concourse.mybir as mybir
from concourse.bass_utils import run_bass_kernel_spmd
from concourse.tile import TileContext

S = 512          # sequence length
B = 256          # total batch
Z = 64           # hidden states
NCORES = 8
Bc = B // NCORES           # batch per core = 32
COLS = S * Bc              # 16384 state columns per core
CH = 64                    # timesteps per e2 chunk
CCOLS = CH * Bc            # 2048 columns per chunk
NCH = S // CH              # 8 chunks
DSL = 256                  # e2 DMA slice columns (8 per chunk)
PCH = 32                   # timesteps per posterior chunk
PCOLS = PCH * Bc           # 1024 posterior chunk columns
NPCH = COLS // PCOLS       # 16 posterior chunks

F32 = mybir.dt.float32
MUL = mybir.AluOpType.mult

_CACHE = {}
LAST_RESULTS = None


def _build_nc():
    nc = bacc.Bacc("TRN2", target_bir_lowering=False, debug=False,
                   num_devices=NCORES)

    e2_d = nc.dram_tensor("e2", [128, COLS], F32, kind="ExternalInput")
    w_d = nc.dram_tensor("w", [128, 128], F32, kind="ExternalInput")
    pi_d = nc.dram_tensor("piext", [128, 1], F32, kind="ExternalInput")
    ones_d = nc.dram_tensor("ones64", [64, 64], F32, kind="ExternalInput")

    alpha_d = nc.dram_tensor("alpha", [64, COLS], F32, kind="ExternalOutput")
    beta_d = nc.dram_tensor("beta", [64, COLS], F32, kind="ExternalOutput")
    post_d = nc.dram_tensor("post", [64, COLS], F32, kind="ExternalOutput")

    with TileContext(nc) as tc:
        with (
            tc.tile_pool(name="const", bufs=1) as constp,
            tc.tile_pool(name="state", bufs=1) as statep,
            tc.tile_pool(name="betap", bufs=1) as betapp,
            tc.tile_pool(name="linv", bufs=1) as linvp,
            tc.tile_pool(name="e2", bufs=2) as e2p,
            tc.tile_pool(name="po", bufs=2) as pop,
            tc.tile_pool(name="pot", bufs=2) as potp,
            tc.tile_pool(name="mm", bufs=4, space="PSUM") as mmp,
            tc.tile_pool(name="aux", bufs=2, space="PSUM") as auxp,
        ):
            # ---- constants ----
            w_t = constp.tile([128, 128], F32, tag="w")
            pi_t = constp.tile([128, 1], F32, tag="pi")
            ones_t = constp.tile([64, 64], F32, tag="ones")
            nc.sync.dma_start(w_t[:], w_d[:])
            nc.sync.dma_start(pi_t[:], pi_d[:])
            nc.sync.dma_start(ones_t[:], ones_d[:])

            state = statep.tile([128, COLS], F32, tag="state")
            betap = betapp.tile([128, COLS], F32, tag="beta")  # rows 64:128 used
            linvt = linvp.tile([64, CCOLS], F32, tag="linv")

            e2tiles = {}

            def issue_e2(c):
                """Stream e2 chunk c from DRAM in DSL-column slices."""
                t = e2p.tile([128, CCOLS], F32, tag="e2", name=f"e2_{c}")
                e2tiles[c] = t
                base = c * CCOLS
                for h in range(CCOLS // DSL):
                    nc.sync.dma_start(
                        t[:, h * DSL:(h + 1) * DSL],
                        e2_d[:, base + h * DSL:base + (h + 1) * DSL])

            # ---- prologue ----
            issue_e2(0)
            issue_e2(1)
            # state col 0 = e2 col 0 * [ones; pi]
            nc.vector.tensor_scalar(state[:, 0:Bc], e2tiles[0][:, 0:Bc],
                                    pi_t[:, 0:1], None, MUL)
            # beta[S-1] = 1
            nc.gpsimd.memset(betap[64:128, (S - 1) * Bc:S * Bc], 1.0)

            # ---- posterior machinery ----
            # L_b = sum_z alpha_t[z,b]*beta_t[z,b] is t-independent; computed
            # at the mid-scan step from column group MIDQ.  Pool computes
            # ab = alpha.*beta for that column group, PE reduces over z with
            # a ones-stationary matmul into 64 identical rows, DVE
            # reciprocates -> linv tile rows, Pool tiles it across CCOLS.
            MIDQ = S // 2 - 1          # both alpha_q and beta_q exist then

            def post_chunk_ops(p):
                """Closures for posterior chunk p (Pool TTs + output DMA),
                sliced in SLC-column pieces."""
                SLC = 512
                ops = []
                po = {}

                def mk(i):
                    def fn():
                        if i == 0:
                            po["t"] = pop.tile([64, PCOLS], F32, tag="po",
                                               name=f"po_{p}")
                        s = slice(p * PCOLS + i * SLC,
                                  p * PCOLS + (i + 1) * SLC)
                        d = slice(i * SLC, (i + 1) * SLC)
                        nc.gpsimd.tensor_tensor(po["t"][:, d], state[64:128, s],
                                                betap[64:128, s], MUL)
                        nc.gpsimd.tensor_tensor(
                            po["t"][:, d], po["t"][:, d],
                            linvt[:, i * SLC % CCOLS:
                                  (i * SLC % CCOLS) + SLC], MUL)
                        if i == PCOLS // SLC - 1:
                            nc.sync.dma_start(
                                post_d[:, p * PCOLS:(p + 1) * PCOLS],
                                po["t"][:])
                    return fn

                for i in range(PCOLS // SLC):
                    ops.append(mk(i))
                return ops

            # Posterior schedule: chunk p needs alpha through t=32p+31
            # (ready after step j=32p+31) and beta through t=32p
            # (beta_tb written at step j=510-tb -> ready after j=510-32p).
            # Both ready at j >= max(32p+31, 510-32p); also needs linv
            # (ready ~ MIDQ+6).  Chunks 0 and 15 are handled by slivers.
            POST_SCHED = {}
            for p in range(1, NPCH - 1):
                j0 = max(32 * p + 33, 512 - 32 * p, MIDQ + 10)
                for i, fn in enumerate(post_chunk_ops(p)):
                    POST_SCHED.setdefault(j0 + 3 * i, []).append(fn)

            # Sliver schedule for chunks 0 (beta arrives last) and 15
            # (alpha arrives last): process 8 timesteps (256 cols) per
            # sliver as soon as their data lands.
            SLV = 8 * Bc
            po_tail = {}

            def sliver(p, k):
                """Posterior for cols [p*PCOLS + k*SLV : +SLV]."""
                def fn():
                    if p not in po_tail:
                        po_tail[p] = potp.tile([64, PCOLS], F32, tag="pot",
                                               name=f"pot_{p}")
                    s = slice(p * PCOLS + k * SLV, p * PCOLS + (k + 1) * SLV)
                    d = slice(k * SLV, (k + 1) * SLV)
                    nc.gpsimd.tensor_tensor(po_tail[p][:, d],
                                            state[64:128, s],
                                            betap[64:128, s], MUL)
                    nc.gpsimd.tensor_tensor(
                        po_tail[p][:, d], po_tail[p][:, d],
                        linvt[:, (p * PCOLS + k * SLV) % CCOLS:
                              (p * PCOLS + k * SLV) % CCOLS + SLV], MUL)
                return fn

            # chunk 15 sliver k covers t=480+8k..487+8k: alpha_t written at
            # step j=t-1 -> ready after j=487+8k (k=3 lands on the last
            # step, j=510).  beta chunk 15 (t=480..511) ready by j=30.
            for k in range(4):
                POST_SCHED.setdefault(min(488 + 8 * k, 510), []).append(
                    sliver(15, k))
            # chunk 0 sliver k covers t=8k..8k+7: beta_t written at step
            # j=510-t -> ready at the end of step j=510-8k; alpha chunk 0
            # is ready early (j=31).
            for k in range(4):
                POST_SCHED.setdefault(510 - 8 * k, []).append(sliver(0, k))

            # ---- merged forward/backward scan ----
            for j in range(S - 1):
                c, off = j // CH, j % CH
                if off == 0 and c + 2 < NCH:
                    issue_e2(c + 2)

                ps = mmp.tile([128, Bc], F32, tag="mm")
                nc.tensor.matmul(ps[:], w_t[:], state[:, j * Bc:(j + 1) * Bc])
                q = j + 1
                nc.vector.tensor_tensor(
                    state[:, q * Bc:(q + 1) * Bc], ps[:],
                    e2tiles[q // CH][:, (q % CH) * Bc:((q % CH) + 1) * Bc],
                    MUL)
                tb = S - 2 - j
                nc.scalar.copy(betap[64:128, tb * Bc:(tb + 1) * Bc], ps[0:64, :])

                # mid-scan normalizer: L columns from group MIDQ
                if j == MIDQ + 2:
                    abm = pop.tile([64, Bc], F32, tag="po", name="abmid")
                    nc.gpsimd.tensor_tensor(
                        abm[:], state[64:128, MIDQ * Bc:(MIDQ + 1) * Bc],
                        betap[64:128, MIDQ * Bc:(MIDQ + 1) * Bc], MUL)
                elif j == MIDQ + 4:
                    lsum = auxp.tile([64, Bc], F32, tag="aux", name="lsum")
                    nc.tensor.matmul(lsum[:], ones_t[:], abm[:])
                elif j == MIDQ + 6:
                    nc.vector.reciprocal(linvt[:, 0:Bc], lsum[:])
                elif j == MIDQ + 8:
                    # tile linv [64, Bc] -> [64, CCOLS] by doubling
                    w_ = Bc
                    while w_ < CCOLS:
                        nc.gpsimd.tensor_copy(linvt[:, w_:min(2 * w_, CCOLS)],
                                              linvt[:, 0:min(w_, CCOLS - w_)])
                        w_ *= 2

                # stream completed alpha/beta chunks out under the scan
                if off == CH - 1 and c < NCH - 1:       # alpha chunk c done
                    acs = slice(c * CCOLS, (c + 1) * CCOLS)
                    nc.sync.dma_start(alpha_d[:, acs], state[64:128, acs])
                bc_ = (S - 1 - j) // CH
                if bc_ >= 1 and j == (S - 1) - CH * bc_ and bc_ <= NCH - 1:
                    bcs = slice(bc_ * CCOLS, (bc_ + 1) * CCOLS)
                    nc.sync.dma_start(beta_d[:, bcs], betap[64:128, bcs])
                for fn in POST_SCHED.get(j, []):
                    fn()

            # ---- epilogue: final output streaming ----
            cs = slice((NCH - 1) * CCOLS, NCH * CCOLS)
            nc.sync.dma_start(alpha_d[:, cs], state[64:128, cs])
            cs = slice(0, CCOLS)
            nc.sync.dma_start(beta_d[:, cs], betap[64:128, cs])
            nc.sync.dma_start(post_d[:, 15 * PCOLS:16 * PCOLS], po_tail[15][:])
            nc.sync.dma_start(post_d[:, 0:PCOLS], po_tail[0][:])

    nc.finalize()
    return nc


def kernel(input, T, pi, emit):
    global LAST_RESULTS
    input = np.asarray(input)
    T = np.asarray(T, dtype=np.float32)
    pi = np.asarray(pi, dtype=np.float32)
    emit = np.asarray(emit, dtype=np.float32)

    if "nc" not in _CACHE:
        _CACHE["nc"] = _build_nc()
    nc = _CACHE["nc"]

    W = np.zeros((128, 128), np.float32)
    W[:64, :64] = T          # backward block: out_top = T^T @ v
    W[64:, 64:] = T.T        # forward block:  out_bot = T @ alpha
    pi_ext = np.ones((128, 1), np.float32)
    pi_ext[64:, 0] = pi
    ones64 = np.ones((64, 64), np.float32)

    in_maps = []
    for c in range(NCORES):
        sl = np.asarray(input[:, c * Bc:(c + 1) * Bc], dtype=np.int64)
        ef = emit[sl]                              # [S, Bc, Z]
        e2 = np.empty((128, COLS), np.float32)
        e2[64:128] = ef.transpose(2, 0, 1).reshape(Z, COLS)
        e2[0:64] = ef[::-1].transpose(2, 0, 1).reshape(Z, COLS)
        in_maps.append({
            "e2": e2,
            "w": W,
            "piext": pi_ext,
            "ones64": ones64,
        })

    res = run_bass_kernel_spmd(nc, in_maps, core_ids=list(range(NCORES)))
    LAST_RESULTS = res

    alpha = np.empty((S, B, Z), np.float32)
    beta = np.empty((S, B, Z), np.float32)
    post = np.empty((S, B, Z), np.float32)
    for c in range(NCORES):
        r = res.results[c]
        bs = slice(c * Bc, (c + 1) * Bc)
        alpha[:, bs, :] = r["alpha"].reshape(Z, S, Bc).transpose(1, 2, 0)
        beta[:, bs, :] = r["beta"].reshape(Z, S, Bc).transpose(1, 2, 0)
        post[:, bs, :] = r["post"].reshape(Z, S, Bc).transpose(1, 2, 0)
    return alpha, beta, post


# revision 12
# speedup vs baseline: 1.1173x; 1.0091x over previous
"""HMM forward-backward (batch=256, seq=512, Z=64) on 8 Trainium2 NeuronCores.

Strategy (data parallel over batch, 32 batch elements per core):
  - Emission rows are pre-gathered ON HOST into the merged layout
    e2[128, S*Bc]: rows 0:64 = emit[input[S-1-k,b]] (backward, time-reversed),
    rows 64:128 = emit[input[k,b]] (forward); column index = k*Bc+b.
    The device streams e2 per 64-step chunk via plain HWDGE DMA (no
    gathers, no PE transposes) in 256-column slices for fine-grained deps.
  - Forward and backward recursions are merged into ONE 128-contraction
    matmul per step with a block-diagonal stationary matrix
    W = diag(T, T^T):  state = [v_{S-1-q} (rows 0:64) ; alpha_q (rows 64:128)]
    per column group q.  One DVE multiply with the e2 column produces the
    next state column.  beta_{S-2-j} is the top PSUM half before the
    multiply and is copied off by the Scalar engine.
    Critical path per step: PE matmul -> DVE tensor_tensor -> PE.
  - posterior = alpha*beta / L where L = sum_z alpha_t*beta_t is CONSTANT
    over t (HMM likelihood identity).  L is computed once mid-scan
    (one ones-vector matmul + reciprocal), broadcast/tiled once, and the
    per-chunk posterior multiplies run on the otherwise-idle GPSIMD (Pool)
    engine so they never touch the PE/DVE critical path.  The two chunks
    whose alpha/beta only complete at scan end are processed in small
    slivers during the last steps to minimize the tail.
  - Outputs are produced in [Z, t*32+b] layout per core; the host
    reassembles/transposes to [S, B, Z] (pure numpy layout work).
"""

import sys

for _p in ("/opt/trn_rl_repo", "/root/.axon_site/_ro/trn_rl_repo"):
    if _p not in sys.path:
        sys.path.append(_p)

import numpy as np

import concourse.bacc as bacc
import concourse.mybir as mybir
from concourse.bass_utils import run_bass_kernel_spmd
from concourse.tile import TileContext

S = 512          # sequence length
B = 256          # total batch
Z = 64           # hidden states
NCORES = 8
Bc = B // NCORES           # batch per core = 32
COLS = S * Bc              # 16384 state columns per core
CH = 64                    # timesteps per e2 chunk
CCOLS = CH * Bc            # 2048 columns per chunk
NCH = S // CH              # 8 chunks
DSL = 256                  # e2 DMA slice columns (8 per chunk)
PCH = 32                   # timesteps per posterior chunk
PCOLS = PCH * Bc           # 1024 posterior chunk columns
NPCH = COLS // PCOLS       # 16 posterior chunks

F32 = mybir.dt.float32
MUL = mybir.AluOpType.mult

_CACHE = {}
LAST_RESULTS = None


def _build_nc():
    nc = bacc.Bacc("TRN2", target_bir_lowering=False, debug=False,
                   num_devices=NCORES)

    e2_d = nc.dram_tensor("e2", [128, COLS], F32, kind="ExternalInput")
    w_d = nc.dram_tensor("w", [128, 128], F32, kind="ExternalInput")
    s0_d = nc.dram_tensor("state0", [128, Bc], F32, kind="ExternalInput")
    ones_d = nc.dram_tensor("ones64", [64, 64], F32, kind="ExternalInput")

    alpha_d = nc.dram_tensor("alpha", [64, COLS], F32, kind="ExternalOutput")
    beta_d = nc.dram_tensor("beta", [64, COLS], F32, kind="ExternalOutput")
    post_d = nc.dram_tensor("post", [64, COLS], F32, kind="ExternalOutput")

    with TileContext(nc) as tc:
        with (
            tc.tile_pool(name="const", bufs=1) as constp,
            tc.tile_pool(name="state", bufs=1) as statep,
            tc.tile_pool(name="betap", bufs=1) as betapp,
            tc.tile_pool(name="linv", bufs=1) as linvp,
            tc.tile_pool(name="e2", bufs=2) as e2p,
            tc.tile_pool(name="po", bufs=2) as pop,
            tc.tile_pool(name="pot", bufs=2) as potp,
            tc.tile_pool(name="mm", bufs=4, space="PSUM") as mmp,
            tc.tile_pool(name="aux", bufs=2, space="PSUM") as auxp,
        ):
            # ---- constants (w and host-computed state0 first: the first
            # chain matmul needs only these two DMAs) ----
            w_t = constp.tile([128, 128], F32, tag="w")
            ones_t = constp.tile([64, 64], F32, tag="ones")

            state = statep.tile([128, COLS], F32, tag="state")
            betap = betapp.tile([128, COLS], F32, tag="beta")  # rows 64:128 used
            linvt = linvp.tile([64, CCOLS], F32, tag="linv")

            nc.sync.dma_start(w_t[:], w_d[:])
            nc.sync.dma_start(state[:, 0:Bc], s0_d[:])

            e2tiles = {}

            def issue_e2(c):
                """Stream e2 chunk c from DRAM in DSL-column slices."""
                t = e2p.tile([128, CCOLS], F32, tag="e2", name=f"e2_{c}")
                e2tiles[c] = t
                base = c * CCOLS
                for h in range(CCOLS // DSL):
                    nc.sync.dma_start(
                        t[:, h * DSL:(h + 1) * DSL],
                        e2_d[:, base + h * DSL:base + (h + 1) * DSL])

            # ---- prologue ----
            issue_e2(0)
            nc.sync.dma_start(ones_t[:], ones_d[:])
            issue_e2(1)
            # beta[S-1] = 1
            nc.gpsimd.memset(betap[64:128, (S - 1) * Bc:S * Bc], 1.0)

            # ---- posterior machinery ----
            # L_b = sum_z alpha_t[z,b]*beta_t[z,b] is t-independent; computed
            # at the mid-scan step from column group MIDQ.  Pool computes
            # ab = alpha.*beta for that column group, PE reduces over z with
            # a ones-stationary matmul into 64 identical rows, DVE
            # reciprocates -> linv tile rows, Pool tiles it across CCOLS.
            MIDQ = S // 2 - 1          # both alpha_q and beta_q exist then

            def post_chunk_ops(p):
                """Closures for posterior chunk p (Pool TTs + output DMA),
                sliced in SLC-column pieces."""
                SLC = 512
                ops = []
                po = {}

                def mk(i):
                    def fn():
                        if i == 0:
                            po["t"] = pop.tile([64, PCOLS], F32, tag="po",
                                               name=f"po_{p}")
                        s = slice(p * PCOLS + i * SLC,
                                  p * PCOLS + (i + 1) * SLC)
                        d = slice(i * SLC, (i + 1) * SLC)
                        nc.gpsimd.tensor_tensor(po["t"][:, d], state[64:128, s],
                                                betap[64:128, s], MUL)
                        nc.gpsimd.tensor_tensor(
                            po["t"][:, d], po["t"][:, d],
                            linvt[:, i * SLC % CCOLS:
                                  (i * SLC % CCOLS) + SLC], MUL)
                        if i == PCOLS // SLC - 1:
                            nc.sync.dma_start(
                                post_d[:, p * PCOLS:(p + 1) * PCOLS],
                                po["t"][:])
                    return fn

                for i in range(PCOLS // SLC):
                    ops.append(mk(i))
                return ops

            # Posterior schedule: chunk p needs alpha through t=32p+31
            # (ready after step j=32p+31) and beta through t=32p
            # (beta_tb written at step j=510-tb -> ready after j=510-32p).
            # Both ready at j >= max(32p+31, 510-32p); also needs linv
            # (ready ~ MIDQ+6).  Chunks 0 and 15 are handled by slivers.
            POST_SCHED = {}
            for p in range(1, NPCH - 1):
                j0 = max(32 * p + 33, 512 - 32 * p, MIDQ + 10)
                for i, fn in enumerate(post_chunk_ops(p)):
                    POST_SCHED.setdefault(j0 + 3 * i, []).append(fn)

            # Sliver schedule for chunks 0 (beta arrives last) and 15
            # (alpha arrives last): process 8 timesteps (256 cols) per
            # sliver as soon as their data lands.
            SLV = 8 * Bc
            po_tail = {}

            def sliver(p, k):
                """Posterior for cols [p*PCOLS + k*SLV : +SLV]."""
                def fn():
                    if p not in po_tail:
                        po_tail[p] = potp.tile([64, PCOLS], F32, tag="pot",
                                               name=f"pot_{p}")
                    s = slice(p * PCOLS + k * SLV, p * PCOLS + (k + 1) * SLV)
                    d = slice(k * SLV, (k + 1) * SLV)
                    nc.gpsimd.tensor_tensor(po_tail[p][:, d],
                                            state[64:128, s],
                                            betap[64:128, s], MUL)
                    nc.gpsimd.tensor_tensor(
                        po_tail[p][:, d], po_tail[p][:, d],
                        linvt[:, (p * PCOLS + k * SLV) % CCOLS:
                              (p * PCOLS + k * SLV) % CCOLS + SLV], MUL)
                return fn

            # chunk 15 sliver k covers t=480+8k..487+8k: alpha_t written at
            # step j=t-1 -> ready after j=487+8k (k=3 lands on the last
            # step, j=510).  beta chunk 15 (t=480..511) ready by j=30.
            for k in range(4):
                POST_SCHED.setdefault(min(488 + 8 * k, 510), []).append(
                    sliver(15, k))
            # chunk 0 sliver k covers t=8k..8k+7: beta_t written at step
            # j=510-t -> ready at the end of step j=510-8k; alpha chunk 0
            # is ready early (j=31).
            for k in range(4):
                POST_SCHED.setdefault(510 - 8 * k, []).append(sliver(0, k))

            # ---- merged forward/backward scan ----
            for j in range(S - 1):
                c, off = j // CH, j % CH
                if off == 0 and c + 2 < NCH:
                    issue_e2(c + 2)

                ps = mmp.tile([128, Bc], F32, tag="mm")
                nc.tensor.matmul(ps[:], w_t[:], state[:, j * Bc:(j + 1) * Bc])
                q = j + 1
                nc.vector.tensor_tensor(
                    state[:, q * Bc:(q + 1) * Bc], ps[:],
                    e2tiles[q // CH][:, (q % CH) * Bc:((q % CH) + 1) * Bc],
                    MUL)
                tb = S - 2 - j
                nc.scalar.copy(betap[64:128, tb * Bc:(tb + 1) * Bc], ps[0:64, :])

                # mid-scan normalizer: L columns from group MIDQ
                if j == MIDQ + 2:
                    abm = pop.tile([64, Bc], F32, tag="po", name="abmid")
                    nc.gpsimd.tensor_tensor(
                        abm[:], state[64:128, MIDQ * Bc:(MIDQ + 1) * Bc],
                        betap[64:128, MIDQ * Bc:(MIDQ + 1) * Bc], MUL)
                elif j == MIDQ + 4:
                    lsum = auxp.tile([64, Bc], F32, tag="aux", name="lsum")
                    nc.tensor.matmul(lsum[:], ones_t[:], abm[:])
                elif j == MIDQ + 6:
                    nc.vector.reciprocal(linvt[:, 0:Bc], lsum[:])
                elif j == MIDQ + 8:
                    # tile linv [64, Bc] -> [64, CCOLS] by doubling
                    w_ = Bc
                    while w_ < CCOLS:
                        nc.gpsimd.tensor_copy(linvt[:, w_:min(2 * w_, CCOLS)],
                                              linvt[:, 0:min(w_, CCOLS - w_)])
                        w_ *= 2

                # stream completed alpha/beta chunks out under the scan
                if off == CH - 1 and c < NCH - 1:       # alpha chunk c done
                    acs = slice(c * CCOLS, (c + 1) * CCOLS)
                    nc.sync.dma_start(alpha_d[:, acs], state[64:128, acs])
                bc_ = (S - 1 - j) // CH
                if bc_ >= 1 and j == (S - 1) - CH * bc_ and bc_ <= NCH - 1:
                    bcs = slice(bc_ * CCOLS, (bc_ + 1) * CCOLS)
                    nc.sync.dma_start(beta_d[:, bcs], betap[64:128, bcs])
                if j == 479:
                    # first halves of the last alpha chunk (t=448..479,
                    # written by j=478) and of beta chunk 0 (t=32..63,
                    # written by j=478)
                    acs = slice(448 * Bc, 480 * Bc)
                    nc.sync.dma_start(alpha_d[:, acs], state[64:128, acs])
                    bcs = slice(32 * Bc, 64 * Bc)
                    nc.sync.dma_start(beta_d[:, bcs], betap[64:128, bcs])
                elif j == 497:
                    # posterior chunk 15 first half (t=480..495, slivers
                    # k=0,1 done by j=496)
                    nc.sync.dma_start(post_d[:, 480 * Bc:496 * Bc],
                                      po_tail[15][:, 0:16 * Bc])
                elif j == 495:
                    # posterior chunk 0 second half (t=16..31, slivers
                    # k=2,3 done by j=494)
                    nc.sync.dma_start(post_d[:, 16 * Bc:32 * Bc],
                                      po_tail[0][:, 16 * Bc:32 * Bc])
                for fn in POST_SCHED.get(j, []):
                    fn()

            # ---- epilogue: final output streaming (small dependency-last
            # pieces only) ----
            cs = slice(480 * Bc, 512 * Bc)
            nc.sync.dma_start(alpha_d[:, cs], state[64:128, cs])
            cs = slice(0, 32 * Bc)
            nc.sync.dma_start(beta_d[:, cs], betap[64:128, cs])
            nc.sync.dma_start(post_d[:, 496 * Bc:512 * Bc],
                              po_tail[15][:, 16 * Bc:32 * Bc])
            nc.sync.dma_start(post_d[:, 0:16 * Bc], po_tail[0][:, 0:16 * Bc])

    nc.finalize()
    return nc


def kernel(input, T, pi, emit):
    global LAST_RESULTS
    input = np.asarray(input)
    T = np.asarray(T, dtype=np.float32)
    pi = np.asarray(pi, dtype=np.float32)
    emit = np.asarray(emit, dtype=np.float32)

    if "nc" not in _CACHE:
        _CACHE["nc"] = _build_nc()
    nc = _CACHE["nc"]

    W = np.zeros((128, 128), np.float32)
    W[:64, :64] = T          # backward block: out_top = T^T @ v
    W[64:, 64:] = T.T        # forward block:  out_bot = T @ alpha
    pi_ext = np.ones((128, 1), np.float32)
    pi_ext[64:, 0] = pi
    ones64 = np.ones((64, 64), np.float32)

    in_maps = []
    for c in range(NCORES):
        sl = np.asarray(input[:, c * Bc:(c + 1) * Bc], dtype=np.int64)
        ef = emit[sl]                              # [S, Bc, Z]
        e2 = np.empty((128, COLS), np.float32)
        e2[64:128] = ef.transpose(2, 0, 1).reshape(Z, COLS)
        e2[0:64] = ef[::-1].transpose(2, 0, 1).reshape(Z, COLS)
        in_maps.append({
            "e2": e2,
            "w": W,
            "state0": e2[:, 0:Bc] * pi_ext,
            "ones64": ones64,
        })

    res = run_bass_kernel_spmd(nc, in_maps, core_ids=list(range(NCORES)))
    LAST_RESULTS = res

    alpha = np.empty((S, B, Z), np.float32)
    beta = np.empty((S, B, Z), np.float32)
    post = np.empty((S, B, Z), np.float32)
    for c in range(NCORES):
        r = res.results[c]
        bs = slice(c * Bc, (c + 1) * Bc)
        alpha[:, bs, :] = r["alpha"].reshape(Z, S, Bc).transpose(1, 2, 0)
        beta[:, bs, :] = r["beta"].reshape(Z, S, Bc).transpose(1, 2, 0)
        post[:, bs, :] = r["post"].reshape(Z, S, Bc).transpose(1, 2, 0)
    return alpha, beta, post


# revision 14
# speedup vs baseline: 1.1195x; 1.0020x over previous
"""HMM forward-backward (batch=256, seq=512, Z=64) on 8 Trainium2 NeuronCores.

Strategy (data parallel over batch, 32 batch elements per core):
  - Emission rows are pre-gathered ON HOST into the merged layout
    e2[128, S*Bc]: rows 0:64 = emit[input[S-1-k,b]] (backward, time-reversed),
    rows 64:128 = emit[input[k,b]] (forward); column index = k*Bc+b.
    The device streams e2 per 64-step chunk via plain HWDGE DMA (no
    gathers, no PE transposes) in 256-column slices for fine-grained deps.
  - Forward and backward recursions are merged into ONE 128-contraction
    matmul per step with a block-diagonal stationary matrix
    W = diag(T, T^T):  state = [v_{S-1-q} (rows 0:64) ; alpha_q (rows 64:128)]
    per column group q.  One DVE multiply with the e2 column produces the
    next state column.  beta_{S-2-j} is the top PSUM half before the
    multiply and is copied off by the Scalar engine.
    Critical path per step: PE matmul -> DVE tensor_tensor -> PE.
  - posterior = alpha*beta / L where L = sum_z alpha_t*beta_t is CONSTANT
    over t (HMM likelihood identity).  L is computed once mid-scan
    (one ones-vector matmul + reciprocal), broadcast/tiled once, and the
    per-chunk posterior multiplies run on the otherwise-idle GPSIMD (Pool)
    engine so they never touch the PE/DVE critical path.  The two chunks
    whose alpha/beta only complete at scan end are processed in small
    slivers during the last steps to minimize the tail.
  - Outputs are produced in [Z, t*32+b] layout per core; the host
    reassembles/transposes to [S, B, Z] (pure numpy layout work).
"""

import sys

for _p in ("/opt/trn_rl_repo", "/root/.axon_site/_ro/trn_rl_repo"):
    if _p not in sys.path:
        sys.path.append(_p)

import numpy as np

import concourse.bacc as bacc
import concourse.mybir as mybir
from concourse.bass_utils import run_bass_kernel_spmd
from concourse.tile import TileContext

S = 512          # sequence length
B = 256          # total batch
Z = 64           # hidden states
NCORES = 8
Bc = B // NCORES           # batch per core = 32
COLS = S * Bc              # 16384 state columns per core
CH = 64                    # timesteps per e2 chunk
CCOLS = CH * Bc            # 2048 columns per chunk
NCH = S // CH              # 8 chunks
DSL = 256                  # e2 DMA slice columns (8 per chunk)
PCH = 32                   # timesteps per posterior chunk
PCOLS = PCH * Bc           # 1024 posterior chunk columns
NPCH = COLS // PCOLS       # 16 posterior chunks

F32 = mybir.dt.float32
MUL = mybir.AluOpType.mult

_CACHE = {}
LAST_RESULTS = None


def _build_nc():
    nc = bacc.Bacc("TRN2", target_bir_lowering=False, debug=False,
                   num_devices=NCORES)

    e2_d = nc.dram_tensor("e2", [128, COLS], F32, kind="ExternalInput")
    w_d = nc.dram_tensor("w", [128, 128], F32, kind="ExternalInput")
    s0_d = nc.dram_tensor("state0", [128, Bc], F32, kind="ExternalInput")
    ones_d = nc.dram_tensor("ones64", [64, 64], F32, kind="ExternalInput")

    alpha_d = nc.dram_tensor("alpha", [64, COLS], F32, kind="ExternalOutput")
    beta_d = nc.dram_tensor("beta", [64, COLS], F32, kind="ExternalOutput")
    post_d = nc.dram_tensor("post", [64, COLS], F32, kind="ExternalOutput")

    with TileContext(nc) as tc:
        with (
            tc.tile_pool(name="const", bufs=1) as constp,
            tc.tile_pool(name="state", bufs=1) as statep,
            tc.tile_pool(name="betap", bufs=1) as betapp,
            tc.tile_pool(name="linv", bufs=1) as linvp,
            tc.tile_pool(name="e2", bufs=2) as e2p,
            tc.tile_pool(name="po", bufs=2) as pop,
            tc.tile_pool(name="pot", bufs=2) as potp,
            tc.tile_pool(name="mm", bufs=4, space="PSUM") as mmp,
            tc.tile_pool(name="aux", bufs=2, space="PSUM") as auxp,
        ):
            # ---- constants (w and host-computed state0 first: the first
            # chain matmul needs only these two DMAs) ----
            w_t = constp.tile([128, 128], F32, tag="w")
            ones_t = constp.tile([64, 64], F32, tag="ones")

            state = statep.tile([128, COLS], F32, tag="state")
            betap = betapp.tile([128, COLS], F32, tag="beta")  # rows 64:128 used
            linvt = linvp.tile([64, CCOLS], F32, tag="linv")

            nc.sync.dma_start(w_t[:], w_d[:])
            nc.scalar.dma_start(state[:, 0:Bc], s0_d[:])
            nc.scalar.dma_start(ones_t[:], ones_d[:])

            e2tiles = {}

            def issue_e2(c):
                """Stream e2 chunk c from DRAM in DSL-column slices."""
                t = e2p.tile([128, CCOLS], F32, tag="e2", name=f"e2_{c}")
                e2tiles[c] = t
                base = c * CCOLS
                for h in range(CCOLS // DSL):
                    nc.sync.dma_start(
                        t[:, h * DSL:(h + 1) * DSL],
                        e2_d[:, base + h * DSL:base + (h + 1) * DSL])

            # ---- prologue ----
            issue_e2(0)
            issue_e2(1)
            # beta[S-1] = 1
            nc.gpsimd.memset(betap[64:128, (S - 1) * Bc:S * Bc], 1.0)

            # ---- posterior machinery ----
            # L_b = sum_z alpha_t[z,b]*beta_t[z,b] is t-independent; computed
            # at the mid-scan step from column group MIDQ.  Pool computes
            # ab = alpha.*beta for that column group, PE reduces over z with
            # a ones-stationary matmul into 64 identical rows, DVE
            # reciprocates -> linv tile rows, Pool tiles it across CCOLS.
            MIDQ = S // 2 - 1          # both alpha_q and beta_q exist then

            def post_chunk_ops(p):
                """Closures for posterior chunk p (Pool TTs + output DMA),
                sliced in SLC-column pieces."""
                SLC = 512
                ops = []
                po = {}

                def mk(i):
                    def fn():
                        if i == 0:
                            po["t"] = pop.tile([64, PCOLS], F32, tag="po",
                                               name=f"po_{p}")
                        s = slice(p * PCOLS + i * SLC,
                                  p * PCOLS + (i + 1) * SLC)
                        d = slice(i * SLC, (i + 1) * SLC)
                        nc.gpsimd.tensor_tensor(po["t"][:, d], state[64:128, s],
                                                betap[64:128, s], MUL)
                        nc.gpsimd.tensor_tensor(
                            po["t"][:, d], po["t"][:, d],
                            linvt[:, i * SLC % CCOLS:
                                  (i * SLC % CCOLS) + SLC], MUL)
                        if i == PCOLS // SLC - 1:
                            nc.sync.dma_start(
                                post_d[:, p * PCOLS:(p + 1) * PCOLS],
                                po["t"][:])
                    return fn

                for i in range(PCOLS // SLC):
                    ops.append(mk(i))
                return ops

            # Posterior schedule: chunk p needs alpha through t=32p+31
            # (ready after step j=32p+31) and beta through t=32p
            # (beta_tb written at step j=510-tb -> ready after j=510-32p).
            # Both ready at j >= max(32p+31, 510-32p); also needs linv
            # (ready ~ MIDQ+6).  Chunks 0 and 15 are handled by slivers.
            POST_SCHED = {}
            for p in range(1, NPCH - 1):
                j0 = max(32 * p + 33, 512 - 32 * p, MIDQ + 10)
                for i, fn in enumerate(post_chunk_ops(p)):
                    POST_SCHED.setdefault(j0 + 3 * i, []).append(fn)

            # Sliver schedule for chunks 0 (beta arrives last) and 15
            # (alpha arrives last): process 8 timesteps (256 cols) per
            # sliver as soon as their data lands.
            SLV = 8 * Bc
            po_tail = {}

            def sliver(p, k, eng=None):
                """Posterior for cols [p*PCOLS + k*SLV : +SLV]."""
                def fn():
                    e = eng if eng is not None else nc.gpsimd
                    if p not in po_tail:
                        po_tail[p] = potp.tile([64, PCOLS], F32, tag="pot",
                                               name=f"pot_{p}")
                    s = slice(p * PCOLS + k * SLV, p * PCOLS + (k + 1) * SLV)
                    d = slice(k * SLV, (k + 1) * SLV)
                    e.tensor_tensor(po_tail[p][:, d],
                                    state[64:128, s],
                                    betap[64:128, s], MUL)
                    e.tensor_tensor(
                        po_tail[p][:, d], po_tail[p][:, d],
                        linvt[:, (p * PCOLS + k * SLV) % CCOLS:
                              (p * PCOLS + k * SLV) % CCOLS + SLV], MUL)
                return fn

            # chunk 15 sliver k covers t=480+8k..487+8k: alpha_t written at
            # step j=t-1 -> ready after j=487+8k (k=3 lands on the last
            # step, j=510).  beta chunk 15 (t=480..511) ready by j=30.
            for k in range(4):
                POST_SCHED.setdefault(min(488 + 8 * k, 510), []).append(
                    sliver(15, k, eng=nc.vector if k == 3 else None))
            # chunk 0 sliver k covers t=8k..8k+7: beta_t written at step
            # j=510-t -> ready at the end of step j=510-8k; alpha chunk 0
            # is ready early (j=31).
            for k in range(4):
                POST_SCHED.setdefault(510 - 8 * k, []).append(sliver(0, k))

            # ---- merged forward/backward scan ----
            for j in range(S - 1):
                c, off = j // CH, j % CH
                if off == 0 and c + 2 < NCH:
                    issue_e2(c + 2)

                ps = mmp.tile([128, Bc], F32, tag="mm")
                nc.tensor.matmul(ps[:], w_t[:], state[:, j * Bc:(j + 1) * Bc])
                q = j + 1
                nc.vector.tensor_tensor(
                    state[:, q * Bc:(q + 1) * Bc], ps[:],
                    e2tiles[q // CH][:, (q % CH) * Bc:((q % CH) + 1) * Bc],
                    MUL)
                tb = S - 2 - j
                nc.scalar.copy(betap[64:128, tb * Bc:(tb + 1) * Bc], ps[0:64, :])

                # mid-scan normalizer: L columns from group MIDQ
                if j == MIDQ + 2:
                    abm = pop.tile([64, Bc], F32, tag="po", name="abmid")
                    nc.gpsimd.tensor_tensor(
                        abm[:], state[64:128, MIDQ * Bc:(MIDQ + 1) * Bc],
                        betap[64:128, MIDQ * Bc:(MIDQ + 1) * Bc], MUL)
                elif j == MIDQ + 4:
                    lsum = auxp.tile([64, Bc], F32, tag="aux", name="lsum")
                    nc.tensor.matmul(lsum[:], ones_t[:], abm[:])
                elif j == MIDQ + 6:
                    nc.vector.reciprocal(linvt[:, 0:Bc], lsum[:])
                elif j == MIDQ + 8:
                    # tile linv [64, Bc] -> [64, CCOLS] by doubling
                    w_ = Bc
                    while w_ < CCOLS:
                        nc.gpsimd.tensor_copy(linvt[:, w_:min(2 * w_, CCOLS)],
                                              linvt[:, 0:min(w_, CCOLS - w_)])
                        w_ *= 2

                # stream completed alpha/beta chunks out under the scan
                if off == CH - 1 and c < NCH - 1:       # alpha chunk c done
                    acs = slice(c * CCOLS, (c + 1) * CCOLS)
                    nc.sync.dma_start(alpha_d[:, acs], state[64:128, acs])
                bc_ = (S - 1 - j) // CH
                if bc_ >= 1 and j == (S - 1) - CH * bc_ and bc_ <= NCH - 1:
                    bcs = slice(bc_ * CCOLS, (bc_ + 1) * CCOLS)
                    nc.sync.dma_start(beta_d[:, bcs], betap[64:128, bcs])
                if j == 479:
                    # first halves of the last alpha chunk (t=448..479,
                    # written by j=478) and of beta chunk 0 (t=32..63,
                    # written by j=478)
                    acs = slice(448 * Bc, 480 * Bc)
                    nc.sync.dma_start(alpha_d[:, acs], state[64:128, acs])
                    bcs = slice(32 * Bc, 64 * Bc)
                    nc.sync.dma_start(beta_d[:, bcs], betap[64:128, bcs])
                elif j == 497:
                    # posterior chunk 15 first half (t=480..495, slivers
                    # k=0,1 done by j=496)
                    nc.sync.dma_start(post_d[:, 480 * Bc:496 * Bc],
                                      po_tail[15][:, 0:16 * Bc])
                elif j == 495:
                    # posterior chunk 0 second half (t=16..31, slivers
                    # k=2,3 done by j=494)
                    nc.sync.dma_start(post_d[:, 16 * Bc:32 * Bc],
                                      po_tail[0][:, 16 * Bc:32 * Bc])
                for fn in POST_SCHED.get(j, []):
                    fn()

            # ---- epilogue: final output streaming (small dependency-last
            # pieces only) ----
            cs = slice(480 * Bc, 512 * Bc)
            nc.sync.dma_start(alpha_d[:, cs], state[64:128, cs])
            cs = slice(0, 32 * Bc)
            nc.scalar.dma_start(beta_d[:, cs], betap[64:128, cs])
            nc.scalar.dma_start(post_d[:, 496 * Bc:512 * Bc],
                                po_tail[15][:, 16 * Bc:32 * Bc])
            nc.sync.dma_start(post_d[:, 0:16 * Bc], po_tail[0][:, 0:16 * Bc])

    nc.finalize()
    return nc


def kernel(input, T, pi, emit):
    global LAST_RESULTS
    input = np.asarray(input)
    T = np.asarray(T, dtype=np.float32)
    pi = np.asarray(pi, dtype=np.float32)
    emit = np.asarray(emit, dtype=np.float32)

    if "nc" not in _CACHE:
        _CACHE["nc"] = _build_nc()
    nc = _CACHE["nc"]

    W = np.zeros((128, 128), np.float32)
    W[:64, :64] = T          # backward block: out_top = T^T @ v
    W[64:, 64:] = T.T        # forward block:  out_bot = T @ alpha
    pi_ext = np.ones((128, 1), np.float32)
    pi_ext[64:, 0] = pi
    ones64 = np.ones((64, 64), np.float32)

    in_maps = []
    for c in range(NCORES):
        sl = np.asarray(input[:, c * Bc:(c + 1) * Bc], dtype=np.int64)
        ef = emit[sl]                              # [S, Bc, Z]
        e2 = np.empty((128, COLS), np.float32)
        e2[64:128] = ef.transpose(2, 0, 1).reshape(Z, COLS)
        e2[0:64] = ef[::-1].transpose(2, 0, 1).reshape(Z, COLS)
        in_maps.append({
            "e2": e2,
            "w": W,
            "state0": e2[:, 0:Bc] * pi_ext,
            "ones64": ones64,
        })

    res = run_bass_kernel_spmd(nc, in_maps, core_ids=list(range(NCORES)))
    LAST_RESULTS = res

    alpha = np.empty((S, B, Z), np.float32)
    beta = np.empty((S, B, Z), np.float32)
    post = np.empty((S, B, Z), np.float32)
    for c in range(NCORES):
        r = res.results[c]
        bs = slice(c * Bc, (c + 1) * Bc)
        alpha[:, bs, :] = r["alpha"].reshape(Z, S, Bc).transpose(1, 2, 0)
        beta[:, bs, :] = r["beta"].reshape(Z, S, Bc).transpose(1, 2, 0)
        post[:, bs, :] = r["post"].reshape(Z, S, Bc).transpose(1, 2, 0)
    return alpha, beta, post


# revision 15
# speedup vs baseline: 1.1506x; 1.0278x over previous
"""HMM forward-backward (batch=256, seq=512, Z=64) on 8 Trainium2 NeuronCores.

Strategy (data parallel over batch, 32 batch elements per core):
  - Emission rows are pre-gathered ON HOST into the merged layout
    e2[128, S*Bc]: rows 0:64 = emit[input[S-1-k,b]] (backward, time-reversed),
    rows 64:128 = emit[input[k,b]] (forward); column index = k*Bc+b.
    pi is baked into columns 0:Bc so state column 0 is a plain DMA of
    e2[:, 0:Bc] (via the Pool SWDGE queue, parallel to the HWDGE line).
    The device streams e2 per 64-step chunk via plain HWDGE DMA (no
    gathers, no PE transposes) in 256-column slices for fine-grained deps.
  - Forward and backward recursions are merged into ONE 128-contraction
    matmul per step with a block-diagonal stationary matrix
    W = diag(T, T^T):  state = [v_{S-1-q} (rows 0:64) ; alpha_q (rows 64:128)]
    per column group q.  beta_{S-2-j} is the top PSUM half before the
    emission multiply and is copied off by the Scalar engine.
    Each step is split into two 16-column half-chains (L/R) that ping-pong
    PE and DVE half a period apart: the DVE tensor_tensor's exec time
    halves while its fixed PSUM-access latency overlaps the other half.
  - posterior = alpha*beta / L where L = sum_z alpha_t*beta_t is CONSTANT
    over t (HMM likelihood identity).  L is computed once mid-scan
    (one ones-vector matmul + reciprocal), tiled once, and the per-chunk
    posterior multiplies run on the otherwise-idle GPSIMD (Pool) engine so
    they never touch the PE/DVE critical path.  The two chunks whose
    alpha/beta only complete at scan end use pre-multiplied factors
    (al0 = alpha*Linv, bl15 = beta*Linv) so each end-of-scan sliver is a
    single tensor_tensor, and outputs stream out in small late pieces
    spread across the SP and Activation DMA queues.
  - Outputs are produced in [Z, t*32+b] layout per core; the host
    reassembles/transposes to [S, B, Z] (pure numpy layout work).
"""

import sys

for _p in ("/opt/trn_rl_repo", "/root/.axon_site/_ro/trn_rl_repo"):
    if _p not in sys.path:
        sys.path.append(_p)

import numpy as np

import concourse.bacc as bacc
import concourse.mybir as mybir
from concourse.bass_utils import run_bass_kernel_spmd
from concourse.tile import TileContext

S = 512          # sequence length
B = 256          # total batch
Z = 64           # hidden states
NCORES = 8
Bc = B // NCORES           # batch per core = 32
HB = Bc // 2               # half-chain columns = 16
COLS = S * Bc              # 16384 state columns per core
CH = 64                    # timesteps per e2 chunk
CCOLS = CH * Bc            # 2048 columns per chunk
NCH = S // CH              # 8 chunks
DSL = 256                  # e2 DMA slice columns (8 per chunk)
PCH = 32                   # timesteps per posterior chunk
PCOLS = PCH * Bc           # 1024 posterior chunk columns
NPCH = COLS // PCOLS       # 16 posterior chunks

F32 = mybir.dt.float32
MUL = mybir.AluOpType.mult

_CACHE = {}
LAST_RESULTS = None


def _build_nc():
    nc = bacc.Bacc("TRN2", target_bir_lowering=False, debug=False,
                   num_devices=NCORES)

    e2_d = nc.dram_tensor("e2", [128, COLS], F32, kind="ExternalInput")
    w_d = nc.dram_tensor("w", [128, 128], F32, kind="ExternalInput")
    ones_d = nc.dram_tensor("ones64", [64, 64], F32, kind="ExternalInput")

    alpha_d = nc.dram_tensor("alpha", [64, COLS], F32, kind="ExternalOutput")
    beta_d = nc.dram_tensor("beta", [64, COLS], F32, kind="ExternalOutput")
    post_d = nc.dram_tensor("post", [64, COLS], F32, kind="ExternalOutput")

    with TileContext(nc) as tc:
        with (
            tc.tile_pool(name="const", bufs=1) as constp,
            tc.tile_pool(name="state", bufs=1) as statep,
            tc.tile_pool(name="betap", bufs=1) as betapp,
            tc.tile_pool(name="linv", bufs=1) as linvp,
            tc.tile_pool(name="pre", bufs=2) as prep,
            tc.tile_pool(name="e2", bufs=2) as e2p,
            tc.tile_pool(name="po", bufs=2) as pop,
            tc.tile_pool(name="pot", bufs=2) as potp,
            tc.tile_pool(name="mm", bufs=6, space="PSUM") as mmp,
            tc.tile_pool(name="aux", bufs=2, space="PSUM") as auxp,
        ):
            w_t = constp.tile([128, 128], F32, tag="w")
            ones_t = constp.tile([64, 64], F32, tag="ones")

            state = statep.tile([128, COLS], F32, tag="state")
            betap = betapp.tile([128, COLS], F32, tag="beta")  # rows 64:128
            # Linv replicated on BOTH partition halves so it can pair with
            # base-0 (po tiles) and base-64 (state/betap) operands.
            linvt = linvp.tile([128, CCOLS], F32, tag="linv")

            # w first on the HWDGE line (gates the first matmul); state
            # col 0 = e2[:, 0:Bc] (pi baked on host) via the parallel
            # Pool SWDGE queue.
            nc.sync.dma_start(w_t[:], w_d[:])
            nc.gpsimd.dma_start(state[:, 0:Bc], e2_d[:, 0:Bc])

            e2tiles = {}

            def issue_e2(c):
                """Stream e2 chunk c from DRAM in DSL-column slices."""
                t = e2p.tile([128, CCOLS], F32, tag="e2", name=f"e2_{c}")
                e2tiles[c] = t
                base = c * CCOLS
                for h in range(CCOLS // DSL):
                    nc.sync.dma_start(
                        t[:, h * DSL:(h + 1) * DSL],
                        e2_d[:, base + h * DSL:base + (h + 1) * DSL])

            # ---- prologue ----
            issue_e2(0)
            nc.scalar.dma_start(ones_t[:], ones_d[:])
            issue_e2(1)
            # beta[S-1] = 1
            nc.gpsimd.memset(betap[64:128, (S - 1) * Bc:S * Bc], 1.0)

            # ---- posterior machinery ----
            # L_b = sum_z alpha_t[z,b]*beta_t[z,b] is t-independent
            # (HMM likelihood identity); computed once mid-scan.
            MIDQ = S // 2 - 1          # both alpha_q and beta_q exist then

            def post_chunk_ops(p):
                """Closures for posterior chunk p (Pool TTs + output DMA),
                sliced in SLC-column pieces."""
                SLC = 512
                ops = []
                po = {}

                def mk(i):
                    def fn():
                        if i == 0:
                            po["t"] = pop.tile([64, PCOLS], F32, tag="po",
                                               name=f"po_{p}")
                        s = slice(p * PCOLS + i * SLC,
                                  p * PCOLS + (i + 1) * SLC)
                        d = slice(i * SLC, (i + 1) * SLC)
                        nc.gpsimd.tensor_tensor(po["t"][:, d], state[64:128, s],
                                                betap[64:128, s], MUL)
                        nc.gpsimd.tensor_tensor(po["t"][:, d], po["t"][:, d],
                                                linvt[0:64, i * SLC:
                                                      i * SLC + SLC], MUL)
                        if i == PCOLS // SLC - 1:
                            nc.sync.dma_start(
                                post_d[:, p * PCOLS:(p + 1) * PCOLS],
                                po["t"][:])
                    return fn

                for i in range(PCOLS // SLC):
                    ops.append(mk(i))
                return ops

            POST_SCHED = {}
            for p in range(1, NPCH - 1):
                j0 = max(32 * p + 33, 512 - 32 * p, MIDQ + 10)
                for i, fn in enumerate(post_chunk_ops(p)):
                    POST_SCHED.setdefault(j0 + 3 * i, []).append(fn)

            # Tail chunks 0 (beta arrives last) and 15 (alpha arrives last):
            # pre-multiplied factors al0 = alpha*Linv, bl15 = beta*Linv make
            # each sliver a single TT against the late-arriving operand.
            SLV = 8 * Bc
            pre_t = {}
            po_tail = {}

            def make_pre():
                pre_t["al0"] = prep.tile([128, PCOLS], F32, tag="pre",
                                         name="al0")
                pre_t["bl15"] = prep.tile([128, PCOLS], F32, tag="pre",
                                          name="bl15")
                nc.gpsimd.tensor_tensor(pre_t["al0"][64:128, :],
                                        state[64:128, 0:PCOLS],
                                        linvt[64:128, 0:PCOLS], MUL)
                nc.gpsimd.tensor_tensor(pre_t["bl15"][64:128, :],
                                        betap[64:128, 15 * PCOLS:16 * PCOLS],
                                        linvt[64:128, 0:PCOLS], MUL)

            def sliver(p, k, eng=None):
                """Posterior for cols [p*PCOLS + k*SLV : +SLV]: one TT of the
                late operand against the pre-multiplied factor."""
                def fn():
                    e = eng if eng is not None else nc.gpsimd
                    if p not in po_tail:
                        po_tail[p] = potp.tile([64, PCOLS], F32, tag="pot",
                                               name=f"pot_{p}")
                    s = slice(p * PCOLS + k * SLV, p * PCOLS + (k + 1) * SLV)
                    d = slice(k * SLV, (k + 1) * SLV)
                    late = state if p == 15 else betap
                    fac = pre_t["bl15"] if p == 15 else pre_t["al0"]
                    e.tensor_tensor(po_tail[p][:, d], late[64:128, s],
                                    fac[64:128, d], MUL)
                return fn

            # chunk 15 sliver k covers t=480+8k..487+8k: alpha ready after
            # step j=487+8k (k=3 on the last step, on the by-then-idle DVE).
            for k in range(4):
                POST_SCHED.setdefault(min(488 + 8 * k, 510), []).append(
                    sliver(15, k, eng=nc.vector if k == 3 else None))
            # chunk 0 sliver k covers t=8k..8k+7: beta ready at the end of
            # step j=510-8k.
            for k in range(4):
                POST_SCHED.setdefault(510 - 8 * k, []).append(sliver(0, k))

            # ---- merged forward/backward scan, two half-chains per step ----
            for j in range(S - 1):
                c, off = j // CH, j % CH
                if off == 0 and c + 2 < NCH:
                    issue_e2(c + 2)

                q = j + 1
                e2t = e2tiles[q // CH]
                eb = (q % CH) * Bc
                psh = []
                for h in range(2):
                    ps = mmp.tile([128, HB], F32, tag="mm")
                    psh.append(ps)
                    nc.tensor.matmul(
                        ps[:], w_t[:],
                        state[:, j * Bc + h * HB:j * Bc + (h + 1) * HB])
                for h in range(2):
                    nc.vector.tensor_tensor(
                        state[:, q * Bc + h * HB:q * Bc + (h + 1) * HB],
                        psh[h][:], e2t[:, eb + h * HB:eb + (h + 1) * HB], MUL)
                tb = S - 2 - j
                for h in range(2):
                    nc.scalar.copy(
                        betap[64:128, tb * Bc + h * HB:tb * Bc + (h + 1) * HB],
                        psh[h][0:64, :])

                # mid-scan normalizer: L columns from group MIDQ
                if j == MIDQ + 2:
                    abm = pop.tile([64, Bc], F32, tag="po", name="abmid")
                    nc.gpsimd.tensor_tensor(
                        abm[:], state[64:128, MIDQ * Bc:(MIDQ + 1) * Bc],
                        betap[64:128, MIDQ * Bc:(MIDQ + 1) * Bc], MUL)
                elif j == MIDQ + 4:
                    lsum = auxp.tile([64, Bc], F32, tag="aux", name="lsum")
                    nc.tensor.matmul(lsum[:], ones_t[:], abm[:])
                elif j == MIDQ + 6:
                    nc.vector.reciprocal(linvt[0:64, 0:Bc], lsum[:])
                    nc.vector.reciprocal(linvt[64:128, 0:Bc], lsum[:])
                elif j == MIDQ + 8:
                    # tile linv [128, Bc] -> [128, CCOLS] by doubling
                    w_ = Bc
                    while w_ < CCOLS:
                        nc.gpsimd.tensor_copy(linvt[:, w_:min(2 * w_, CCOLS)],
                                              linvt[:, 0:min(w_, CCOLS - w_)])
                        w_ *= 2
                elif j == MIDQ + 16:
                    make_pre()

                # stream completed alpha/beta chunks out under the scan
                if off == CH - 1 and c < NCH - 1:       # alpha chunk c done
                    acs = slice(c * CCOLS, (c + 1) * CCOLS)
                    nc.sync.dma_start(alpha_d[:, acs], state[64:128, acs])
                bc_ = (S - 1 - j) // CH
                if bc_ >= 1 and j == (S - 1) - CH * bc_ and bc_ <= NCH - 1:
                    bcs = slice(bc_ * CCOLS, (bc_ + 1) * CCOLS)
                    nc.sync.dma_start(beta_d[:, bcs], betap[64:128, bcs])
                if j == 479:
                    # early halves of the last alpha chunk (t=448..479) and
                    # of beta chunk 0 (t=32..63), both written by j=478
                    acs = slice(448 * Bc, 480 * Bc)
                    nc.sync.dma_start(alpha_d[:, acs], state[64:128, acs])
                    bcs = slice(32 * Bc, 64 * Bc)
                    nc.sync.dma_start(beta_d[:, bcs], betap[64:128, bcs])
                elif j == 497:
                    # posterior chunk 15 cols 0:512 (slivers k=0,1 by j=496)
                    nc.sync.dma_start(post_d[:, 480 * Bc:496 * Bc],
                                      po_tail[15][:, 0:16 * Bc])
                elif j == 505:
                    # posterior chunk 15 cols 512:768 (sliver k=2 at j=504)
                    nc.sync.dma_start(post_d[:, 496 * Bc:504 * Bc],
                                      po_tail[15][:, 16 * Bc:24 * Bc])
                elif j == 495:
                    # posterior chunk 0 cols 512:1024 (slivers k=2,3 by 494)
                    nc.sync.dma_start(post_d[:, 16 * Bc:32 * Bc],
                                      po_tail[0][:, 16 * Bc:32 * Bc])
                elif j == 503:
                    # posterior chunk 0 cols 256:512 (sliver k=1 at j=502);
                    # beta t=8..31 (written by j=502)
                    nc.sync.dma_start(post_d[:, 8 * Bc:16 * Bc],
                                      po_tail[0][:, 8 * Bc:16 * Bc])
                    nc.sync.dma_start(beta_d[:, 8 * Bc:32 * Bc],
                                      betap[64:128, 8 * Bc:32 * Bc])
                elif j == 508:
                    # alpha t=480..508 written by j=507
                    nc.sync.dma_start(alpha_d[:, 480 * Bc:508 * Bc],
                                      state[64:128, 480 * Bc:508 * Bc])
                for fn in POST_SCHED.get(j, []):
                    fn()

            # ---- epilogue: small dependency-last pieces on two queues ----
            nc.sync.dma_start(alpha_d[:, 508 * Bc:512 * Bc],
                              state[64:128, 508 * Bc:512 * Bc])
            nc.sync.dma_start(post_d[:, 0:8 * Bc], po_tail[0][:, 0:8 * Bc])
            nc.scalar.dma_start(beta_d[:, 0:8 * Bc], betap[64:128, 0:8 * Bc])
            nc.scalar.dma_start(post_d[:, 504 * Bc:512 * Bc],
                                po_tail[15][:, 24 * Bc:32 * Bc])

    nc.finalize()
    return nc


def kernel(input, T, pi, emit):
    global LAST_RESULTS
    input = np.asarray(input)
    T = np.asarray(T, dtype=np.float32)
    pi = np.asarray(pi, dtype=np.float32)
    emit = np.asarray(emit, dtype=np.float32)

    if "nc" not in _CACHE:
        _CACHE["nc"] = _build_nc()
    nc = _CACHE["nc"]

    W = np.zeros((128, 128), np.float32)
    W[:64, :64] = T          # backward block: out_top = T^T @ v
    W[64:, 64:] = T.T        # forward block:  out_bot = T @ alpha
    ones64 = np.ones((64, 64), np.float32)

    in_maps = []
    for c in range(NCORES):
        sl = np.asarray(input[:, c * Bc:(c + 1) * Bc], dtype=np.int64)
        ef = emit[sl]                              # [S, Bc, Z]
        e2 = np.empty((128, COLS), np.float32)
        e2[64:128] = ef.transpose(2, 0, 1).reshape(Z, COLS)
        e2[0:64] = ef[::-1].transpose(2, 0, 1).reshape(Z, COLS)
        e2[64:128, 0:Bc] *= pi[:, None]            # bake pi into state col 0
        in_maps.append({
            "e2": e2,
            "w": W,
            "ones64": ones64,
        })

    res = run_bass_kernel_spmd(nc, in_maps, core_ids=list(range(NCORES)))
    LAST_RESULTS = res

    alpha = np.empty((S, B, Z), np.float32)
    beta = np.empty((S, B, Z), np.float32)
    post = np.empty((S, B, Z), np.float32)
    for c in range(NCORES):
        r = res.results[c]
        bs = slice(c * Bc, (c + 1) * Bc)
        alpha[:, bs, :] = r["alpha"].reshape(Z, S, Bc).transpose(1, 2, 0)
        beta[:, bs, :] = r["beta"].reshape(Z, S, Bc).transpose(1, 2, 0)
        post[:, bs, :] = r["post"].reshape(Z, S, Bc).transpose(1, 2, 0)
    return alpha, beta, post


# revision 16
# speedup vs baseline: 1.1562x; 1.0049x over previous
"""HMM forward-backward (batch=256, seq=512, Z=64) on 8 Trainium2 NeuronCores.

Strategy (data parallel over batch, 32 batch elements per core):
  - Emission rows are pre-gathered ON HOST into the merged layout
    e2[128, S*Bc]: rows 0:64 = emit[input[S-1-k,b]] (backward, time-reversed),
    rows 64:128 = emit[input[k,b]] (forward); column index = k*Bc+b.
    pi is baked into columns 0:Bc so state column 0 is a plain DMA of
    e2[:, 0:Bc] (via the Pool SWDGE queue, parallel to the HWDGE line).
    The device streams e2 per 64-step chunk via plain HWDGE DMA (no
    gathers, no PE transposes) in 256-column slices for fine-grained deps.
  - Forward and backward recursions are merged into ONE 128-contraction
    matmul per step with a block-diagonal stationary matrix
    W = diag(T, T^T):  state = [v_{S-1-q} (rows 0:64) ; alpha_q (rows 64:128)]
    per column group q.  beta_{S-2-j} is the top PSUM half before the
    emission multiply and is copied off by the Scalar engine.
    Each step is split into two 16-column half-chains (L/R) that ping-pong
    PE and DVE half a period apart: the DVE tensor_tensor's exec time
    halves while its fixed PSUM-access latency overlaps the other half.
  - posterior = alpha*beta / L where L = sum_z alpha_t*beta_t is CONSTANT
    over t (HMM likelihood identity).  L is computed once mid-scan
    (one ones-vector matmul + reciprocal), tiled once, and the per-chunk
    posterior multiplies run on the otherwise-idle GPSIMD (Pool) engine so
    they never touch the PE/DVE critical path.  The two chunks whose
    alpha/beta only complete at scan end use pre-multiplied factors
    (al0 = alpha*Linv, bl15 = beta*Linv) so each end-of-scan sliver is a
    single tensor_tensor, and outputs stream out in small late pieces
    spread across the SP and Activation DMA queues.
  - Outputs are produced in [Z, t*32+b] layout per core; the host
    reassembles/transposes to [S, B, Z] (pure numpy layout work).
"""

import sys

for _p in ("/opt/trn_rl_repo", "/root/.axon_site/_ro/trn_rl_repo"):
    if _p not in sys.path:
        sys.path.append(_p)

import numpy as np

import concourse.bacc as bacc
import concourse.mybir as mybir
from concourse.bass_utils import run_bass_kernel_spmd
from concourse.tile import TileContext

S = 512          # sequence length
B = 256          # total batch
Z = 64           # hidden states
NCORES = 8
Bc = B // NCORES           # batch per core = 32
HB = Bc // 2               # half-chain columns = 16
COLS = S * Bc              # 16384 state columns per core
CH = 64                    # timesteps per e2 chunk
CCOLS = CH * Bc            # 2048 columns per chunk
NCH = S // CH              # 8 chunks
DSL = 256                  # e2 DMA slice columns (8 per chunk)
PCH = 32                   # timesteps per posterior chunk
PCOLS = PCH * Bc           # 1024 posterior chunk columns
NPCH = COLS // PCOLS       # 16 posterior chunks

F32 = mybir.dt.float32
MUL = mybir.AluOpType.mult

_CACHE = {}
LAST_RESULTS = None


def _build_nc():
    nc = bacc.Bacc("TRN2", target_bir_lowering=False, debug=False,
                   num_devices=NCORES)

    e2_d = nc.dram_tensor("e2", [128, COLS], F32, kind="ExternalInput")
    w_d = nc.dram_tensor("w", [128, 128 + Bc], F32, kind="ExternalInput")
    ones_d = nc.dram_tensor("ones64", [64, 64], F32, kind="ExternalInput")

    alpha_d = nc.dram_tensor("alpha", [64, COLS], F32, kind="ExternalOutput")
    beta_d = nc.dram_tensor("beta", [64, COLS], F32, kind="ExternalOutput")
    post_d = nc.dram_tensor("post", [64, COLS], F32, kind="ExternalOutput")

    with TileContext(nc) as tc:
        with (
            tc.tile_pool(name="const", bufs=1) as constp,
            tc.tile_pool(name="state", bufs=1) as statep,
            tc.tile_pool(name="betap", bufs=1) as betapp,
            tc.tile_pool(name="linv", bufs=1) as linvp,
            tc.tile_pool(name="pre", bufs=2) as prep,
            tc.tile_pool(name="e2", bufs=2) as e2p,
            tc.tile_pool(name="po", bufs=2) as pop,
            tc.tile_pool(name="pot", bufs=2) as potp,
            tc.tile_pool(name="mm", bufs=6, space="PSUM") as mmp,
            tc.tile_pool(name="aux", bufs=2, space="PSUM") as auxp,
        ):
            # w_t packs [W | state column 0] so one DMA gates the
            # first matmul (stationary = cols 0:128, moving = cols 128:160)
            w_t = constp.tile([128, 128 + Bc], F32, tag="w")
            ones_t = constp.tile([64, 64], F32, tag="ones")

            state = statep.tile([128, COLS], F32, tag="state")
            betap = betapp.tile([128, COLS], F32, tag="beta")  # rows 64:128
            # Linv replicated on BOTH partition halves so it can pair with
            # base-0 (po tiles) and base-64 (state/betap) operands.
            linvt = linvp.tile([128, CCOLS], F32, tag="linv")

            nc.sync.dma_start(w_t[:], w_d[:])
            # later readers (alpha chunk-0 DMA, al0) see state col 0 here:
            nc.gpsimd.tensor_copy(state[:, 0:Bc], w_t[:, 128:128 + Bc])

            e2tiles = {}

            def issue_e2(c):
                """Stream e2 chunk c from DRAM in DSL-column slices."""
                t = e2p.tile([128, CCOLS], F32, tag="e2", name=f"e2_{c}")
                e2tiles[c] = t
                base = c * CCOLS
                cuts = [0, 64, DSL] if c == 0 else [0, DSL]
                cuts = cuts + list(range(2 * DSL, CCOLS + 1, DSL))
                for a, b in zip(cuts, cuts[1:]):
                    nc.sync.dma_start(t[:, a:b], e2_d[:, base + a:base + b])

            # ---- prologue ----
            issue_e2(0)
            issue_e2(1)
            nc.sync.dma_start(ones_t[:], ones_d[:])
            # beta[S-1] = 1
            nc.gpsimd.memset(betap[64:128, (S - 1) * Bc:S * Bc], 1.0)

            # ---- posterior machinery ----
            # L_b = sum_z alpha_t[z,b]*beta_t[z,b] is t-independent
            # (HMM likelihood identity); computed once mid-scan.
            MIDQ = S // 2 - 1          # both alpha_q and beta_q exist then

            def post_chunk_ops(p):
                """Closures for posterior chunk p (Pool TTs + output DMA),
                sliced in SLC-column pieces."""
                SLC = 512
                ops = []
                po = {}

                def mk(i):
                    def fn():
                        if i == 0:
                            po["t"] = pop.tile([64, PCOLS], F32, tag="po",
                                               name=f"po_{p}")
                        s = slice(p * PCOLS + i * SLC,
                                  p * PCOLS + (i + 1) * SLC)
                        d = slice(i * SLC, (i + 1) * SLC)
                        nc.gpsimd.tensor_tensor(po["t"][:, d], state[64:128, s],
                                                betap[64:128, s], MUL)
                        nc.gpsimd.tensor_tensor(po["t"][:, d], po["t"][:, d],
                                                linvt[0:64, i * SLC:
                                                      i * SLC + SLC], MUL)
                        if i == PCOLS // SLC - 1:
                            nc.sync.dma_start(
                                post_d[:, p * PCOLS:(p + 1) * PCOLS],
                                po["t"][:])
                    return fn

                for i in range(PCOLS // SLC):
                    ops.append(mk(i))
                return ops

            POST_SCHED = {}
            for p in range(1, NPCH - 1):
                j0 = max(32 * p + 33, 512 - 32 * p, MIDQ + 10)
                for i, fn in enumerate(post_chunk_ops(p)):
                    POST_SCHED.setdefault(j0 + 3 * i, []).append(fn)

            # Tail chunks 0 (beta arrives last) and 15 (alpha arrives last):
            # pre-multiplied factors al0 = alpha*Linv, bl15 = beta*Linv make
            # each sliver a single TT against the late-arriving operand.
            SLV = 8 * Bc
            pre_t = {}
            po_tail = {}

            def make_pre():
                pre_t["al0"] = prep.tile([128, PCOLS], F32, tag="pre",
                                         name="al0")
                pre_t["bl15"] = prep.tile([128, PCOLS], F32, tag="pre",
                                          name="bl15")
                nc.gpsimd.tensor_tensor(pre_t["al0"][64:128, :],
                                        state[64:128, 0:PCOLS],
                                        linvt[64:128, 0:PCOLS], MUL)
                nc.gpsimd.tensor_tensor(pre_t["bl15"][64:128, :],
                                        betap[64:128, 15 * PCOLS:16 * PCOLS],
                                        linvt[64:128, 0:PCOLS], MUL)

            def sliver(p, k, eng=None):
                """Posterior for cols [p*PCOLS + k*SLV : +SLV]: one TT of the
                late operand against the pre-multiplied factor."""
                def fn():
                    e = eng if eng is not None else nc.gpsimd
                    if p not in po_tail:
                        po_tail[p] = potp.tile([64, PCOLS], F32, tag="pot",
                                               name=f"pot_{p}")
                    s = slice(p * PCOLS + k * SLV, p * PCOLS + (k + 1) * SLV)
                    d = slice(k * SLV, (k + 1) * SLV)
                    late = state if p == 15 else betap
                    fac = pre_t["bl15"] if p == 15 else pre_t["al0"]
                    e.tensor_tensor(po_tail[p][:, d], late[64:128, s],
                                    fac[64:128, d], MUL)
                return fn

            # chunk 15 sliver k covers t=480+8k..487+8k: alpha ready after
            # step j=487+8k (k=3 on the last step, on the by-then-idle DVE).
            for k in range(4):
                POST_SCHED.setdefault(min(488 + 8 * k, 510), []).append(
                    sliver(15, k, eng=nc.vector if k == 3 else None))
            # chunk 0 sliver k covers t=8k..8k+7: beta ready at the end of
            # step j=510-8k.
            for k in range(4):
                POST_SCHED.setdefault(510 - 8 * k, []).append(
                    sliver(0, k, eng=nc.vector if k == 0 else None))

            # ---- merged forward/backward scan, two half-chains per step ----
            for j in range(S - 1):
                c, off = j // CH, j % CH
                if off == 0 and c + 2 < NCH:
                    issue_e2(c + 2)

                q = j + 1
                e2t = e2tiles[q // CH]
                eb = (q % CH) * Bc
                psh = []
                for h in range(2):
                    ps = mmp.tile([128, HB], F32, tag="mm")
                    psh.append(ps)
                    mov = (w_t[:, 128 + h * HB:128 + (h + 1) * HB] if j == 0
                           else state[:, j * Bc + h * HB:j * Bc + (h + 1) * HB])
                    nc.tensor.matmul(ps[:], w_t[:, 0:128], mov)
                for h in range(2):
                    nc.vector.tensor_tensor(
                        state[:, q * Bc + h * HB:q * Bc + (h + 1) * HB],
                        psh[h][:], e2t[:, eb + h * HB:eb + (h + 1) * HB], MUL)
                tb = S - 2 - j
                for h in range(2):
                    nc.scalar.copy(
                        betap[64:128, tb * Bc + h * HB:tb * Bc + (h + 1) * HB],
                        psh[h][0:64, :])

                # mid-scan normalizer: L columns from group MIDQ
                if j == MIDQ + 2:
                    abm = pop.tile([64, Bc], F32, tag="po", name="abmid")
                    nc.gpsimd.tensor_tensor(
                        abm[:], state[64:128, MIDQ * Bc:(MIDQ + 1) * Bc],
                        betap[64:128, MIDQ * Bc:(MIDQ + 1) * Bc], MUL)
                elif j == MIDQ + 4:
                    lsum = auxp.tile([64, Bc], F32, tag="aux", name="lsum")
                    nc.tensor.matmul(lsum[:], ones_t[:], abm[:])
                elif j == MIDQ + 6:
                    nc.vector.reciprocal(linvt[0:64, 0:Bc], lsum[:])
                    nc.vector.reciprocal(linvt[64:128, 0:Bc], lsum[:])
                elif j == MIDQ + 8:
                    # tile linv [128, Bc] -> [128, CCOLS] by doubling
                    w_ = Bc
                    while w_ < CCOLS:
                        nc.gpsimd.tensor_copy(linvt[:, w_:min(2 * w_, CCOLS)],
                                              linvt[:, 0:min(w_, CCOLS - w_)])
                        w_ *= 2
                elif j == MIDQ + 16:
                    make_pre()

                # stream completed alpha/beta chunks out under the scan
                if off == CH - 1 and c < NCH - 1:       # alpha chunk c done
                    acs = slice(c * CCOLS, (c + 1) * CCOLS)
                    nc.sync.dma_start(alpha_d[:, acs], state[64:128, acs])
                bc_ = (S - 1 - j) // CH
                if bc_ >= 1 and j == (S - 1) - CH * bc_ and bc_ <= NCH - 1:
                    bcs = slice(bc_ * CCOLS, (bc_ + 1) * CCOLS)
                    nc.sync.dma_start(beta_d[:, bcs], betap[64:128, bcs])
                if j == 479:
                    # early halves of the last alpha chunk (t=448..479) and
                    # of beta chunk 0 (t=32..63), both written by j=478
                    acs = slice(448 * Bc, 480 * Bc)
                    nc.sync.dma_start(alpha_d[:, acs], state[64:128, acs])
                    bcs = slice(32 * Bc, 64 * Bc)
                    nc.sync.dma_start(beta_d[:, bcs], betap[64:128, bcs])
                elif j == 497:
                    # posterior chunk 15 cols 0:512 (slivers k=0,1 by j=496)
                    nc.sync.dma_start(post_d[:, 480 * Bc:496 * Bc],
                                      po_tail[15][:, 0:16 * Bc])
                elif j == 505:
                    # posterior chunk 15 cols 512:768 (sliver k=2 at j=504)
                    nc.sync.dma_start(post_d[:, 496 * Bc:504 * Bc],
                                      po_tail[15][:, 16 * Bc:24 * Bc])
                elif j == 495:
                    # posterior chunk 0 cols 512:1024 (slivers k=2,3 by 494)
                    nc.sync.dma_start(post_d[:, 16 * Bc:32 * Bc],
                                      po_tail[0][:, 16 * Bc:32 * Bc])
                elif j == 503:
                    # posterior chunk 0 cols 256:512 (sliver k=1 at j=502);
                    # beta t=8..31 (written by j=502)
                    nc.sync.dma_start(post_d[:, 8 * Bc:16 * Bc],
                                      po_tail[0][:, 8 * Bc:16 * Bc])
                    nc.sync.dma_start(beta_d[:, 8 * Bc:32 * Bc],
                                      betap[64:128, 8 * Bc:32 * Bc])
                for fn in POST_SCHED.get(j, []):
                    fn()

            # ---- epilogue: dependency-last pieces spread over the SP
            # and Act HWDGE queues plus the Pool SWDGE queue ----
            nc.sync.dma_start(alpha_d[:, 480 * Bc:512 * Bc],
                              state[64:128, 480 * Bc:512 * Bc])
            nc.sync.dma_start(post_d[:, 0:8 * Bc], po_tail[0][:, 0:8 * Bc])
            nc.scalar.dma_start(beta_d[:, 0:8 * Bc], betap[64:128, 0:8 * Bc])
            nc.gpsimd.dma_start(post_d[:, 504 * Bc:512 * Bc],
                                po_tail[15][:, 24 * Bc:32 * Bc])

    nc.finalize()
    return nc


def kernel(input, T, pi, emit):
    global LAST_RESULTS
    input = np.asarray(input)
    T = np.asarray(T, dtype=np.float32)
    pi = np.asarray(pi, dtype=np.float32)
    emit = np.asarray(emit, dtype=np.float32)

    if "nc" not in _CACHE:
        _CACHE["nc"] = _build_nc()
    nc = _CACHE["nc"]

    W = np.zeros((128, 128), np.float32)
    W[:64, :64] = T          # backward block: out_top = T^T @ v
    W[64:, 64:] = T.T        # forward block:  out_bot = T @ alpha
    ones64 = np.ones((64, 64), np.float32)

    in_maps = []
    for c in range(NCORES):
        sl = np.asarray(input[:, c * Bc:(c + 1) * Bc], dtype=np.int64)
        ef = emit[sl]                              # [S, Bc, Z]
        e2 = np.empty((128, COLS), np.float32)
        e2[64:128] = ef.transpose(2, 0, 1).reshape(Z, COLS)
        e2[0:64] = ef[::-1].transpose(2, 0, 1).reshape(Z, COLS)
        e2[64:128, 0:Bc] *= pi[:, None]            # bake pi into state col 0
        in_maps.append({
            "e2": e2,
            "w": np.concatenate([W, e2[:, 0:Bc]], axis=1),
            "ones64": ones64,
        })

    res = run_bass_kernel_spmd(nc, in_maps, core_ids=list(range(NCORES)))
    LAST_RESULTS = res

    alpha = np.empty((S, B, Z), np.float32)
    beta = np.empty((S, B, Z), np.float32)
    post = np.empty((S, B, Z), np.float32)
    for c in range(NCORES):
        r = res.results[c]
        bs = slice(c * Bc, (c + 1) * Bc)
        alpha[:, bs, :] = r["alpha"].reshape(Z, S, Bc).transpose(1, 2, 0)
        beta[:, bs, :] = r["beta"].reshape(Z, S, Bc).transpose(1, 2, 0)
        post[:, bs, :] = r["post"].reshape(Z, S, Bc).transpose(1, 2, 0)
    return alpha, beta, post
